# revision 27
# baseline (speedup 1.0000x reference)
"""Trainium2 Bass kernel for DyIntraModalityUpdate (dual gated self-attention).

Strategy
--------
Data-parallel over batch: 16 batches -> 8 NeuronCores x 2 batches, zero
collectives.  Each core processes 4 independent "units" (2 batches x
{v-stream, q-stream}); the only cross-stream coupling is the gates
(v_mean gates q's attention and vice versa), computed per batch before the
per-stream work.

All heavy compute is done in a transposed layout [feature, position]:
  - k/qr projections are computed directly transposed: kqrT[f, r], via
    fp8e4m3 DoubleRow matmuls (2 k-tiles of 128 contracted per instruction,
    ~1.5-2x PE throughput).  Weights are pre-scaled x16 on the host so fp8
    values avoid the subnormal range; the resulting 256x score scale is
    folded into the softmax exp scale.
  - scores: per head pair (2k, 2k+1) the S^T matmuls are emitted
    interleaved; the two heads' lhsT/rhs live at partitions 0-63 / 64-127,
    so the PE row-tiling (tile_position rows 0 and 64) runs both heads'
    matmuls concurrently (~2x).
  - E^T = exp(S^T * 0.125/256) written as fp8e4m3 (attention here is very
    flat - probs ~ 1/768 - so fp8 quantization noise averages out).
  - va is computed in natural layout [position, feature] (fp8 DoubleRow),
    gated, stored as fp8 with a 16.0-column appended so the att-out
    matmul's extra output row yields 16x the softmax denominator
    (compensating the x16 va scale exactly after the reciprocal).
  - att-out O^T = va_ext^T @ E^T via fp8 DoubleRow over position-tile
    pairs.
  - normalization multiplies O^T rows by 1/denominator: the denominator row
    is staged to SBUF f32, inverted with reciprocal_approx_fast (the plain
    InstReciprocal costs ~4.7us on HW), broadcast across 64 partitions with
    gpsimd.partition_broadcast (no DMA round trip), and applied 3 half-heads
    later (depth-3 norm pipeline) so DVE never blocks on the Pool hop.
  - residual add on DVE; the Pool engine runs ONLY partition_broadcast:
    mixing gpsimd op families reloads the Q7 overlay (~8us per switch).
  - final projection stays bf16 (precision: the residual feeds the output
    directly).

HW notes (measured, CoreSim's model differs): exp [128,768] from PSUM is
~0.66us and the ACT stream is NOT the wall-clock pacer; fp8 DoubleRow
matmuls run at ~1 row/cycle (only the halved pass count helps); DVE psum
copies ~0.54us; InstReciprocal ~4.7us regardless of partition count;
cross-engine chains cost ~1us per semaphore wake, so every per-head
dependency is pipelined at least 2-3 heads deep.

Problem constants are hardcoded per the harness contract.
"""

import numpy as np
import ml_dtypes

B, N, D, OUT, H, DH = 16, 768, 512, 512, 8, 64
NCORES, BPC = 8, 2
KT = D // 128          # 4 contraction tiles of 128
KP = KT // 2           # 2 DoubleRow pair-tiles
FC_KQR = (2 * OUT) // 128   # 8 feature chunks for k+qr
OC = OUT // 128        # 4 output chunks
MC = N // 128          # 6 position chunks
MP = MC // 2           # 3 DoubleRow position pairs
HP = H // 2            # 4 head pairs
NSPLIT = ((0, 512), (512, 256))   # psum free-dim splits (bank aligned)
WSCALE = 16.0          # host-side fp8 weight prescale (avoids subnormals)
ESCALE = 0.125 / (WSCALE * WSCALE)  # exp scale absorbing k,qr prescale

_CACHE = {}


def _build_program(skip_b_kq, skip_b_va, skip_b_g, skip_b_o, reps=1):
    import os
    from contextlib import ExitStack

    DIAG = frozenset(
        x for x in os.environ.get("KDIAG", "").split(",") if x
    )  # timing-only ablations; breaks numerics

    import concourse.bass as bass
    import concourse.mybir as mybir
    import concourse.tile as tile
    from concourse import bacc

    dt = mybir.dt
    f32, bf, f8 = dt.float32, dt.bfloat16, dt.float8e4
    AF = mybir.ActivationFunctionType
    OP = mybir.AluOpType
    DR = mybir.MatmulPerfMode.DoubleRow

    nc = bacc.Bacc("TRN2", target_bir_lowering=False, debug=False)

    # ---- DRAM parameters (per-core shard) -------------------------------
    xT_d = nc.declare_dram_parameter("xT", [2, BPC, KT, 128, N], bf, isOutput=False)
    x8_d = nc.declare_dram_parameter("x8", [2, BPC, KT, 128, N], f8, isOutput=False)
    wkq_d = nc.declare_dram_parameter("wkq", [2, KT, 128, 2 * OUT], f8, isOutput=False)
    wva_d = nc.declare_dram_parameter("wva", [2, KT, 128, OUT], f8, isOutput=False)
    wg_d = nc.declare_dram_parameter("wg", [2, KT, 128, OUT], bf, isOutput=False)
    wo_d = nc.declare_dram_parameter("wo", [2, KT, 128, OUT], bf, isOutput=False)
    bkq_d = nc.declare_dram_parameter("bkq", [2, 128, FC_KQR], f32, isOutput=False)
    bva_d = nc.declare_dram_parameter("bva", [2, 1, OUT], f32, isOutput=False)
    bgc_d = nc.declare_dram_parameter("bgc", [2, 128, OC], f32, isOutput=False)
    bgr_d = nc.declare_dram_parameter("bgr", [2, 1, OUT], f32, isOutput=False)
    bo_d = nc.declare_dram_parameter("bo", [2, 128, OC], f32, isOutput=False)
    rms_d = nc.declare_dram_parameter("rms", [2, BPC, 128, 1], f32, isOutput=False)
    out_d = nc.declare_dram_parameter("out", [2, BPC, OC, 128, N], f32, isOutput=True)

    with ExitStack() as ctx:
        tc = ctx.enter_context(tile.TileContext(nc))

        const = ctx.enter_context(tc.tile_pool(name="const", bufs=1))
        xpool = ctx.enter_context(tc.tile_pool(name="xp", bufs=4))
        x8pool = ctx.enter_context(tc.tile_pool(name="x8p", bufs=4))
        kqrp = ctx.enter_context(tc.tile_pool(name="kqrp", bufs=2))
        vap = ctx.enter_context(tc.tile_pool(name="vap", bufs=2))
        ep = ctx.enter_context(tc.tile_pool(name="ep", bufs=3))
        atp = ctx.enter_context(tc.tile_pool(name="atp", bufs=3))
        smal = ctx.enter_context(tc.tile_pool(name="smal", bufs=4))
        up = ctx.enter_context(tc.tile_pool(name="up", bufs=3))
        rbp = ctx.enter_context(tc.tile_pool(name="rbp", bufs=3))
        dramp = ctx.enter_context(tc.tile_pool(name="dramp", bufs=2, space="DRAM"))
        # PSUM: 8 banks.  "pss" 2x[128,768] (4 banks) rotate the score
        # chunks PE->ACT; "pso" 1x (2 banks) holds the att-out accumulator;
        # "psx" 1x (2 banks) serves trans/va/proj/gate matmuls.
        psum = ctx.enter_context(tc.tile_pool(name="psum", bufs=1, space="PSUM"))

        # ---- batch-0 activations first ----------------------------------
        # stream 1 first: the first gate (s=0) needs stream 1's mean, so its
        # x load and reduces lead the startup critical path.  x8 loads go on
        # the ACT hwdge queue so they don't queue behind the bf16 loads.
        x_first, x8_first = [None, None], [None, None]
        for s in (1, 0):
            xt = xpool.tile([128, KT, N], bf, name="x", tag="x")
            nc.sync.dma_start(out=xt, in_=xT_d[s, 0].rearrange("t p n -> p t n"))
            x_first[s] = xt
        for s in (0, 1):
            x8 = x8pool.tile([128, KT, N], f8, name="x8", tag="x8")
            nc.scalar.dma_start(out=x8, in_=x8_d[s, 0].rearrange("t p n -> p t n"))
            x8_first[s] = x8

        rms_all = {}
        for bb in range(BPC):
            for s in range(2):
                rt = const.tile([128, 1], f32, name=f"rms{s}_{bb}")
                nc.sync.dma_start(out=rt, in_=rms_d[s, bb])
                rms_all[(s, bb)] = rt

        # ---- load weights / biases once ---------------------------------
        wkq_sb, wva_sb, wg_sb, wo_sb = [], [], [], []
        bkq_sb, bgc_sb, bo_sb, bva_sb, bgr_sb = [], [], [], [], []
        bgcn_sb, bgrn_sb = [], []
        for s in range(2):
            t_kq = const.tile([128, KT, 2 * OUT], f8, name=f"wkq{s}")
            t_va = const.tile([128, KT, OUT], f8, name=f"wva{s}")
            t_g = const.tile([128, KT, OUT], bf, name=f"wg{s}")
            t_o = const.tile([128, KT, OUT], bf, name=f"wo{s}")
            wkq_sb.append(t_kq)
            wva_sb.append(t_va)
            wg_sb.append(t_g)
            wo_sb.append(t_o)
        for s in range(2):
            if not skip_b_kq:
                t = const.tile([128, FC_KQR], f32, name=f"bkq{s}")
                nc.sync.dma_start(out=t, in_=bkq_d[s])
                bkq_sb.append(t)
            else:
                bkq_sb.append(None)
            if not skip_b_g:
                t = const.tile([128, OC], f32, name=f"bgc{s}")
                nc.sync.dma_start(out=t, in_=bgc_d[s])
                bgc_sb.append(t)
                tn = const.tile([128, OC], f32, name=f"bgcn{s}")
                nc.vector.tensor_scalar_mul(tn, t, -1.0)
                bgcn_sb.append(tn)
                t = const.tile([1, OUT], f32, name=f"bgr{s}")
                nc.sync.dma_start(out=t, in_=bgr_d[s])
                bgr_sb.append(t)
                tn = const.tile([1, OUT], f32, name=f"bgrn{s}")
                nc.vector.tensor_scalar_mul(tn, t, -1.0)
                bgrn_sb.append(tn)
            else:
                bgc_sb.append(None)
                bgcn_sb.append(None)
                bgr_sb.append(None)
                bgrn_sb.append(None)
            if not skip_b_o:
                t = const.tile([128, OC], f32, name=f"bo{s}")
                nc.sync.dma_start(out=t, in_=bo_d[s])
                bo_sb.append(t)
            else:
                bo_sb.append(None)
            if not skip_b_va:
                t = const.tile([1, OUT], f32, name=f"bva{s}")
                nc.sync.dma_start(out=t, in_=bva_d[s])
                bva_sb.append(t)
            else:
                bva_sb.append(None)
        if "nogate" in DIAG:
            const_g2 = const.tile([128, OC], f32, name="cg2")
            nc.vector.memset(const_g2, 1.0)
            const_G = const.tile([128, OUT], bf, name="cG")
            nc.vector.memset(const_G, 1.0)
        nc.gpsimd.dma_start(out=wkq_sb[0], in_=wkq_d[0].rearrange("t p f -> p t f"))
        nc.gpsimd.dma_start(out=wg_sb[0], in_=wg_d[0].rearrange("t p f -> p t f"))
        nc.gpsimd.dma_start(out=wg_sb[1], in_=wg_d[1].rearrange("t p f -> p t f"))
        nc.gpsimd.dma_start(out=wva_sb[0], in_=wva_d[0].rearrange("t p f -> p t f"))
        nc.gpsimd.dma_start(out=wkq_sb[1], in_=wkq_d[1].rearrange("t p f -> p t f"))
        nc.sync.dma_start(out=wva_sb[1], in_=wva_d[1].rearrange("t p f -> p t f"))
        nc.sync.dma_start(out=wo_sb[0], in_=wo_d[0].rearrange("t p f -> p t f"))
        nc.sync.dma_start(out=wo_sb[1], in_=wo_d[1].rearrange("t p f -> p t f"))

        # ---- interleaved per-unit emission ------------------------------

        def gen_prep(rep_i, b, st):
            if rep_i == 0 and b == 0:
                st["x"], st["x8"] = x_first, x8_first
            else:
                st["x"], st["x8"] = [], []
                for s in range(2):
                    xt = xpool.tile([128, KT, N], bf, name="x", tag="x")
                    nc.sync.dma_start(
                        out=xt, in_=xT_d[s, b].rearrange("t p n -> p t n")
                    )
                    st["x"].append(xt)
                for s in range(2):
                    x8 = x8pool.tile([128, KT, N], f8, name="x8", tag="x8")
                    nc.sync.dma_start(
                        out=x8, in_=x8_d[s, b].rearrange("t p n -> p t n")
                    )
                    st["x8"].append(x8)
            yield
            if "nogate" in DIAG:
                yield
                st["gcol"] = [const_g2, const_g2]
                st["G"] = [const_G, const_G]
                return
            x_sb = st["x"]
            mean_sb, rms_sb = {}, {}
            for s in (1, 0):  # stream 1 first: gate s=0 needs mean of 1
                rms_sb[s] = rms_all[(s, b)]
                sums = smal.tile([128, KT], f32, name="sums", tag="sums")
                for kt in range(KT):
                    nc.vector.reduce_sum(
                        out=sums[:, kt : kt + 1],
                        in_=x_sb[s][:, kt, :],
                        axis=mybir.AxisListType.X,
                    )
                mean = smal.tile([128, KT], bf, name="mean", tag="mean")
                nc.vector.tensor_copy(mean, sums)
                mean_sb[s] = mean
            yield
            # row-only gate computation: the per-oc column path (16 tiny PE
            # matmuls + 4 ACT exps + 4 DVE ops per stream) cost ~1us per
            # cross-engine hop on HW.  Compute g and g^2 as rows, then turn
            # g^2 into per-partition columns with one DRAM round trip per
            # batch (latency hidden: prep runs a unit ahead).
            gcol_sb, G_sb = [], []
            g2_dram = dramp.tile([2, OUT], f32, name="g2_dram", tag="g2d")
            for s in range(2):
                o = 1 - s
                # sigmoid via exp (stay in ACT's exp table set): rms_d
                # carries -1/mask_sum, so e = exp(-z) and g = 1 + 1/(1+e)
                pr = psum.tile([1, OUT], f32, name="pr", tag="psx")
                for kt in range(KT):
                    nc.tensor.matmul(
                        pr,
                        lhsT=mean_sb[o][:, kt : kt + 1],
                        rhs=wg_sb[s][:, kt, :],
                        start=(kt == 0),
                        stop=(kt == KT - 1),
                    )
                sig_r = smal.tile([1, OUT], f32, name="sig_r", tag="sig_r", bufs=2)
                if skip_b_g:
                    nc.scalar.activation(
                        out=sig_r, in_=pr, func=AF.Exp, scale=rms_sb[o][0:1, :]
                    )
                else:
                    tmp_r = smal.tile([1, OUT], f32, name="tmp_r", tag="tmp_r", bufs=2)
                    nc.vector.scalar_tensor_tensor(
                        out=tmp_r,
                        in0=pr,
                        scalar=rms_sb[o][0:1, :],
                        in1=bgrn_sb[s],
                        op0=OP.mult,
                        op1=OP.add,
                    )
                    nc.scalar.activation(out=sig_r, in_=tmp_r, func=AF.Exp)
                t1r = smal.tile([1, OUT], f32, name="t1r", tag="t1r", bufs=2)
                nc.vector.tensor_scalar_add(t1r, sig_r, 1.0)
                rr = smal.tile([1, OUT], f32, name="rr", tag="rr", bufs=2)
                nc.vector.reciprocal_approx_fast(out=rr, in_=t1r)
                grow = smal.tile([1, OUT], bf, name="grow", tag="grow", bufs=2)
                nc.vector.tensor_scalar_add(grow, rr, 1.0)
                G = rbp.tile([128, OUT], bf, name="G", tag="G", bufs=2)
                nc.gpsimd.partition_broadcast(G, grow)
                G_sb.append(G)
                g2row = smal.tile([1, OUT], f32, name="g2row", tag="g2row", bufs=2)
                nc.vector.tensor_mul(g2row, grow, grow)
                nc.sync.dma_start(out=g2_dram[s : s + 1], in_=g2row)
                yield
            g2col2 = smal.tile([128, 2, OC], f32, name="g2col2", tag="g2c2", bufs=2)
            nc.sync.dma_start(
                out=g2col2, in_=g2_dram.rearrange("s (oc p) -> p s oc", p=128)
            )
            gcol_sb = [g2col2[:, 0, :], g2col2[:, 1, :]]
            st["gcol"], st["G"] = gcol_sb, G_sb

        def gen_trans(st, s):
            x8 = st["x8"][s]
            gcol_sb = st["gcol"]
            kqr = kqrp.tile([128, FC_KQR, N], bf, name="kqr", tag="kqr")
            st[("kqr", s)] = kqr
            # k/qr chunk pairs in head-pair order so pair-0 scores can start
            # after the first two chunks.  Psum alternates between the two
            # filler slots so the PE->DVE drain ping-pong pipelines.
            for fi, fc in enumerate((0, OC, 1, OC + 1, 2, OC + 2, 3, OC + 3)):
                pt = psum.tile([128, N], f32, name="pt", tag="psx" if fi % 2 else "pso")
                for i in range(KP):
                    for n0, nw in NSPLIT:
                        nc.tensor.matmul(
                            pt[:, n0 : n0 + nw],
                            lhsT=wkq_sb[s][:, 2 * i : 2 * i + 2, fc * 128 : (fc + 1) * 128],
                            rhs=x8[:, 2 * i : 2 * i + 2, n0 : n0 + nw],
                            start=(i == 0),
                            stop=(i == KP - 1),
                            perf_mode=DR,
                        )
                if fc < OC:
                    gsl = gcol_sb[s][:, fc : fc + 1]
                    if skip_b_kq:
                        nc.vector.tensor_scalar_mul(kqr[:, fc, :], pt, gsl)
                    else:
                        bg2 = smal.tile([128, 1], f32, name="bg2", tag="bg2")
                        nc.vector.tensor_mul(bg2, bkq_sb[s][:, fc : fc + 1], gsl)
                        nc.scalar.activation(
                            out=kqr[:, fc, :],
                            in_=pt,
                            func=AF.Identity,
                            bias=bg2,
                            scale=gsl,
                        )
                else:
                    if skip_b_kq:
                        nc.vector.tensor_copy(kqr[:, fc, :], pt)
                    else:
                        nc.scalar.activation(
                            out=kqr[:, fc, :],
                            in_=pt,
                            func=AF.Identity,
                            bias=bkq_sb[s][:, fc : fc + 1],
                        )
                yield

            va = vap.tile([128, MC, H, DH + 2], f8, name="va", tag="va")
            st[("va", s)] = va
            with nc.allow_low_precision("fp8 attention values"):
                nc.vector.memset(va[:, :, :, DH : DH + 1], WSCALE)
                nc.vector.memset(va[:, :, :, DH + 1 : DH + 2], 0.0)
            G_h = st["G"][s].rearrange("p (h d) -> p h d", h=H)
            for mc in range(MC):
                pv = psum.tile([128, OUT], f32, name="pv", tag="psx" if mc % 2 else "pso")
                for i in range(KP):
                    nc.tensor.matmul(
                        pv,
                        lhsT=x8[:, 2 * i : 2 * i + 2, mc * 128 : (mc + 1) * 128],
                        rhs=wva_sb[s][:, 2 * i : 2 * i + 2, :],
                        start=(i == 0),
                        stop=(i == KP - 1),
                        perf_mode=DR,
                    )
                pv_h = pv.rearrange("p (h d) -> p h d", h=H)
                with nc.allow_low_precision("fp8 attention values"):
                    nc.vector.tensor_mul(va[:, mc, :, 0:DH], pv_h, G_h)
                    if not skip_b_va:
                        bgr_row = smal.tile([1, OUT], f32, name="bgr_row", tag="bgrr")
                        nc.vector.tensor_mul(bgr_row, bva_sb[s], st["G"][s][0:1, :])
                        bg = rbp.tile([128, OUT], f32, name="bg", tag="bg")
                        nc.gpsimd.partition_broadcast(bg, bgr_row)
                        nc.vector.tensor_add(
                            va[:, mc, :, 0:DH],
                            va[:, mc, :, 0:DH],
                            bg.rearrange("p (h d) -> p h d", h=H),
                        )
                yield

        def gen_heads(st, s, last=False):
            xt = st["x"][s]
            kqr = st[("kqr", s)]
            at = atp.tile([128, OC, N], bf, name="at", tag="at")
            st[("at", s)] = at

            def emit_scores(hp, e8):
                # both heads of the pair interleaved at mc granularity:
                # head 2hp on PE rows 0-63, head 2hp+1 on rows 64-127 run
                # concurrently (row tiling).
                for mc in range(MC):
                    for half in range(2):
                        po = 64 * half
                        ps_s = psum.tile([128, N], f32, name="ps_s", tag="pss", bufs=2)
                        lhsT = kqr[po : po + 64, hp, mc * 128 : (mc + 1) * 128]
                        for n0, nw in NSPLIT:
                            nc.tensor.matmul(
                                ps_s[:, n0 : n0 + nw],
                                lhsT=lhsT,
                                rhs=kqr[po : po + 64, OC + hp, n0 : n0 + nw],
                                start=True,
                                stop=True,
                            )
                        with nc.allow_low_precision("fp8 attention probs"):
                            if "tinyexp" in DIAG:
                                nc.scalar.activation(
                                    out=e8[:, half, mc, 0:32],
                                    in_=ps_s[:, 0:32],
                                    func=AF.Exp,
                                    scale=ESCALE,
                                )
                            else:
                                nc.scalar.activation(
                                    out=e8[:, half, mc, :],
                                    in_=ps_s,
                                    func=AF.Exp,
                                    scale=ESCALE,
                                )
                    yield

            pending_norm = []  # (hp, po, o_sb, rb) with broadcast in flight

            def emit_norm():
                # at-mul + residual for the oldest pending head; its rb
                # broadcast has been in flight while the next head's att
                # matmuls ran, so DVE never waits on the DMA round trip.
                # residual add on DVE (all-SBUF bf16): Pool must stay
                # pbcast-only — mixing gpsimd op families reloads the Q7
                # overlay per switch, serializing the whole norm chain
                nhp, npo, no_sb, nrb = pending_norm.pop(0)
                if nrb is None:  # KDIAG=nonorm: same DVE volume, no rb dep
                    nc.vector.tensor_copy(
                        at[npo : npo + 64, nhp, :], no_sb[0:DH, :]
                    )
                else:
                    nc.vector.tensor_mul(
                        at[npo : npo + 64, nhp, :], no_sb[0:DH, :], nrb
                    )
                nc.vector.tensor_add(
                    at[npo : npo + 64, nhp, :],
                    at[npo : npo + 64, nhp, :],
                    xt[npo : npo + 64, nhp, :],
                )

            def emit_att(hp, e8):
                va = st[("va", s)]  # created by gen_trans's va section
                if "noatt" in DIAG:
                    for half in range(2):
                        yield
                    return
                for half in range(2):
                    h = 2 * hp + half
                    po = 64 * half
                    po_t = psum.tile([DH + 2, N], f32, name="po_t", tag="pso")
                    for n0, nw in NSPLIT:
                        for i in range(MP):
                            nc.tensor.matmul(
                                po_t[:, n0 : n0 + nw],
                                lhsT=va[:, 2 * i : 2 * i + 2, h, :],
                                rhs=e8[:, half, 2 * i : 2 * i + 2, n0 : n0 + nw],
                                start=(i == 0),
                                stop=(i == MP - 1),
                                perf_mode=DR,
                            )
                    o_sb = rbp.tile([DH + 1, N], bf, name="o_sb", tag="o_sb", bufs=4)
                    nc.vector.tensor_copy(o_sb, po_t[0 : DH + 1, :])
                    if "nonorm" in DIAG:
                        pending_norm.append((hp, po, o_sb, None))
                        if len(pending_norm) > 1:
                            emit_norm()
                        yield
                        continue
                    # InstReciprocal costs ~4.7us on HW; approx_fast (~18
                    # correct bits, one custom-DVE op) is ~5x cheaper and far
                    # exceeds the bf16 precision of the multiply it feeds.
                    # Its fp32 bit-trick seed needs a partition-0 SBUF input:
                    # stage the PSUM denominator row first.
                    den32 = smal.tile([1, N], f32, name="den32", tag="den32", bufs=2)
                    nc.vector.tensor_copy(den32, po_t[DH : DH + 1, :])
                    r_row = smal.tile([1, N], f32, name="r_row", tag="r_row", bufs=3)
                    nc.vector.reciprocal_approx_fast(out=r_row, in_=den32)
                    # broadcast 1/den across 64 partitions on the Pool engine:
                    # no DMA round trip, keeps SP free for bulk loads
                    rb = rbp.tile([64, N], f32, name="rb", tag="rb", bufs=4)
                    nc.gpsimd.partition_broadcast(rb, r_row)
                    # depth-3 norm pipeline: the DVE->Pool->DVE round trip
                    # (recip -> pbcast -> norm-mul) costs ~3.5us in sem-wake
                    # latency on HW; emitting the norm 3 half-heads late keeps
                    # DVE from ever blocking on rb
                    pending_norm.append((hp, po, o_sb, rb))
                    if len(pending_norm) > 3:
                        emit_norm()
                    yield

            # software-pipelined: scores of pair hp, then att of pair hp-1.
            # For the final unit there is no later exp work to hide under, so
            # run att eagerly right after each pair's scores to shorten the
            # drain tail.
            if last:
                for hp in range(HP):
                    e8 = ep.tile([128, 2, MC, N], f8, name="e", tag="e")
                    yield from emit_scores(hp, e8)
                    yield from emit_att(hp, e8)
            else:
                prev_e8 = None
                for hp in range(HP):
                    e8 = ep.tile([128, 2, MC, N], f8, name="e", tag="e")
                    yield from emit_scores(hp, e8)
                    if prev_e8 is not None:
                        yield from emit_att(hp - 1, prev_e8)
                    prev_e8 = e8
                yield from emit_att(HP - 1, prev_e8)
            while pending_norm:
                emit_norm()

        def gen_proj(st, s, b):
            at = st[("at", s)]
            if "noproj" in DIAG or "noatt" in DIAG:
                for oc in range(OC):
                    yield
                return
            for oc in range(OC):
                pu = psum.tile([128, N], f32, name="pu", tag="psx" if oc % 2 else "pso")
                for kt in range(KT):
                    for n0, nw in NSPLIT:
                        nc.tensor.matmul(
                            pu[:, n0 : n0 + nw],
                            lhsT=wo_sb[s][:, kt, oc * 128 : (oc + 1) * 128],
                            rhs=at[:, kt, n0 : n0 + nw],
                            start=(kt == 0),
                            stop=(kt == KT - 1),
                        )
                u_sb = up.tile([128, N], f32, name="u", tag="u")
                if skip_b_o:
                    nc.vector.tensor_copy(u_sb, pu)
                else:
                    nc.vector.tensor_scalar_add(u_sb, pu, bo_sb[s][:, oc : oc + 1])
                nc.sync.dma_start(out=out_d[s, b, oc], in_=u_sb)
                yield

        def drain(g):
            if g is not None:
                for _ in g:
                    pass

        units = [(r, bb, s) for r in range(reps) for bb in range(BPC) for s in range(2)]
        states = {}

        def state_for(r, bb):
            return states.setdefault((r, bb), {})

        from itertools import islice

        st0 = state_for(units[0][0], units[0][1])
        drain(gen_prep(units[0][0], units[0][1], st0))
        # stream unit-0's trans: emit the first two k/qr chunks (pair-0
        # scores inputs), leave the rest as a filler inside its head loop
        tr0 = gen_trans(st0, units[0][2])
        for _ in islice(tr0, 2):
            pass

        pending_proj = None
        pending_heads = {}  # unit index -> (generator, yields already consumed)
        for i, (r, bb, s) in enumerate(units):
            st = state_for(r, bb)
            fillers = []
            if i == 0:
                fillers.append(tr0)
            if pending_proj is not None:
                fillers.append(pending_proj)
            nxt_heads = None
            pre = [0]
            if i + 1 < len(units):
                rn, bn, sn = units[i + 1]
                stn = state_for(rn, bn)
                if (rn, bn) != (r, bb):
                    fillers.append(gen_prep(rn, bn, stn))
                fillers.append(gen_trans(stn, sn))
                # cross-unit head overlap: after the next unit's trans/va
                # fillers drain, let its first score chunks emit inside THIS
                # unit's head loop so ACT's exp stream never drains at the
                # unit boundary

                def counted(g, cnt):
                    for x in g:
                        cnt[0] += 1
                        yield x

                nxt_heads = gen_heads(stn, sn, last=(i + 1 == len(units) - 1))
                fillers.append(islice(counted(nxt_heads, pre), 16))
            heads, done = pending_heads.pop(i, (None, 0))
            if heads is None:
                heads = gen_heads(st, s)
            nyield = HP * MC + H  # score-chunk yields + att-half yields
            for h in range(nyield - done):
                if next(heads, StopIteration) is StopIteration:
                    break
                # drip-feed fillers: 1/yield early so serialized trans/proj
                # drains never dam up the PE stream ahead of score matmuls,
                # 2/yield later to finish the supply before this unit ends
                for _ in range(1 if h < 12 else 2):
                    while fillers:
                        try:
                            next(fillers[0])
                            break
                        except StopIteration:
                            fillers.pop(0)
                    else:
                        break
            drain(heads)
            for g in fillers:
                drain(g)
            if nxt_heads is not None:
                pending_heads[i + 1] = (nxt_heads, pre[0])
            pending_proj = gen_proj(st, s, bb)
        drain(pending_proj)

    nc.finalize()
    return nc


def _prep_inputs(inputs):
    bf16 = ml_dtypes.bfloat16
    fp8 = ml_dtypes.float8_e4m3
    f32 = np.float32

    def arr(name):
        return np.asarray(inputs[name], f32)

    v, q = arr("v"), arr("q")
    v_mask, q_mask = arr("v_mask"), arr("q_mask")

    def prep_x(x, dtype):  # [B, N, D] -> [B, KT, 128, N] (transposed)
        xt = np.ascontiguousarray(x.transpose(0, 2, 1))
        return xt.reshape(B, KT, 128, N).astype(dtype)

    def prep_w(w, dtype, scale=1.0):  # [F, D] -> [KT, 128, F]  (= w.T tiled)
        wt = np.ascontiguousarray(w.T) * f32(scale)
        return wt.reshape(KT, 128, -1).astype(dtype)

    def col128(bias, scale=1.0):  # [F] -> [128, F//128] f32 columns
        return np.ascontiguousarray((bias * f32(scale)).reshape(-1, 128).T).astype(f32)

    w_v, w_q = arr("w_v"), arr("w_q")
    b_v, b_q = arr("b_v"), arr("b_q")
    w_q4v, w_v4q = arr("w_q4v"), arr("w_v4q")
    b_q4v, b_v4q = arr("b_q4v"), arr("b_v4q")
    w_vo, w_qo = arr("w_vo"), arr("w_qo")
    b_vo, b_qo = arr("b_vo"), arr("b_qo")

    xT = np.stack([prep_x(v, bf16), prep_x(q, bf16)])  # [2, B, KT, 128, N]
    x8 = np.stack([prep_x(v, fp8), prep_x(q, fp8)])
    wkq = np.stack(
        [prep_w(w_v[: 2 * OUT], fp8, WSCALE), prep_w(w_q[: 2 * OUT], fp8, WSCALE)]
    )
    wva = np.stack(
        [prep_w(w_v[2 * OUT :], fp8, WSCALE), prep_w(w_q[2 * OUT :], fp8, WSCALE)]
    )
    wg = np.stack([prep_w(w_q4v, bf16), prep_w(w_v4q, bf16)])  # stream 0 gated via q_mean
    wo = np.stack([prep_w(w_vo, bf16), prep_w(w_qo, bf16)])
    bkq = np.stack(
        [col128(b_v[: 2 * OUT], WSCALE), col128(b_q[: 2 * OUT], WSCALE)]
    )
    bva = np.stack(
        [b_v[2 * OUT :][None, :], b_q[2 * OUT :][None, :]]
    ).astype(f32) * f32(WSCALE)
    bgc = np.stack([col128(b_q4v), col128(b_v4q)])
    bgr = np.stack([b_q4v[None, :], b_v4q[None, :]]).astype(f32)
    bo = np.stack([col128(b_vo), col128(b_qo)])

    rms_v = -1.0 / v_mask.sum(1)  # [B]; negative: kernel computes exp(-z)
    rms_q = -1.0 / q_mask.sum(1)
    rms = np.empty((2, B, 128, 1), f32)
    rms[0] = np.broadcast_to(rms_v[:, None, None], (B, 128, 1))
    rms[1] = np.broadcast_to(rms_q[:, None, None], (B, 128, 1))

    skips = (
        bool((b_v[: 2 * OUT] == 0).all() and (b_q[: 2 * OUT] == 0).all()),
        bool((b_v[2 * OUT :] == 0).all() and (b_q[2 * OUT :] == 0).all()),
        bool((b_q4v == 0).all() and (b_v4q == 0).all()),
        bool((b_vo == 0).all() and (b_qo == 0).all()),
    )

    in_maps = []
    for c in range(NCORES):
        sl = slice(c * BPC, (c + 1) * BPC)
        in_maps.append(
            {
                "xT": np.ascontiguousarray(xT[:, sl]),
                "x8": np.ascontiguousarray(x8[:, sl]),
                "wkq": wkq,
                "wva": wva,
                "wg": wg,
                "wo": wo,
                "bkq": bkq,
                "bva": bva,
                "bgc": bgc,
                "bgr": bgr,
                "bo": bo,
                "rms": np.ascontiguousarray(rms[:, sl]),
            }
        )
    return in_maps, skips


def _get_program(skips, reps=1):
    import os

    key = ("prog", skips, reps, os.environ.get("KDIAG", ""))
    if key not in _CACHE:
        _CACHE[key] = _build_program(*skips, reps=reps)
    return _CACHE[key]


def kernel(trace=False, **inputs):
    from concourse.bass_utils import run_bass_kernel_spmd

    in_maps, skips = _prep_inputs(inputs)
    nc = _get_program(skips)
    res = run_bass_kernel_spmd(
        nc, in_maps, core_ids=list(range(NCORES)), trace=trace
    )
    _CACHE["last_result"] = res
    outs = np.stack([r["out"] for r in res.results])  # [8, 2, BPC, OC, 128, N]
    u = outs.reshape(NCORES, 2, BPC, D, N)
    uv = u[:, 0].reshape(B, D, N).transpose(0, 2, 1)
    uq = u[:, 1].reshape(B, D, N).transpose(0, 2, 1)
    return (
        np.ascontiguousarray(uv).astype(np.float32),
        np.ascontiguousarray(uq).astype(np.float32),
    )



# revision 28
# speedup vs baseline: 1.1516x; 1.1516x over previous
"""Trainium2 Bass kernel for DyIntraModalityUpdate (dual gated self-attention).

Strategy
--------
Data-parallel over batch: 16 batches -> 8 NeuronCores x 2 batches, zero
collectives.  Each core processes 4 independent "units" (2 batches x
{v-stream, q-stream}); the only cross-stream coupling is the gates
(v_mean gates q's attention and vice versa), computed per batch before the
per-stream work.

All heavy compute is done in a transposed layout [feature, position]:
  - k/qr projections are computed directly transposed: kqrT[f, r], via
    fp8e4m3 DoubleRow matmuls (2 k-tiles of 128 contracted per instruction,
    ~1.5-2x PE throughput).  Weights are pre-scaled x16 on the host so fp8
    values avoid the subnormal range; the resulting 256x score scale is
    folded into the softmax exp scale.
  - scores: per head pair (2k, 2k+1) the S^T matmuls are emitted
    interleaved; the two heads' lhsT/rhs live at partitions 0-63 / 64-127,
    so the PE row-tiling (tile_position rows 0 and 64) runs both heads'
    matmuls concurrently (~2x).
  - E^T = exp(S^T * 0.125/256) written as fp8e4m3 (attention here is very
    flat - probs ~ 1/768 - so fp8 quantization noise averages out).
  - va is computed in natural layout [position, feature] (fp8 DoubleRow),
    gated, stored as fp8 with a 16.0-column appended so the att-out
    matmul's extra output row yields 16x the softmax denominator
    (compensating the x16 va scale exactly after the reciprocal).
  - att-out O^T = va_ext^T @ E^T via fp8 DoubleRow over position-tile
    pairs.
  - normalization multiplies O^T rows by 1/denominator: the denominator row
    is staged to SBUF f32, inverted with reciprocal_approx_fast (the plain
    InstReciprocal costs ~4.7us on HW), broadcast across 64 partitions with
    gpsimd.partition_broadcast (no DMA round trip), and applied 3 half-heads
    later (depth-3 norm pipeline) so DVE never blocks on the Pool hop.
  - residual add on DVE; the Pool engine runs ONLY partition_broadcast:
    mixing gpsimd op families reloads the Q7 overlay (~8us per switch).
  - final projection stays bf16 (precision: the residual feeds the output
    directly).

HW notes (measured, CoreSim's model differs): exp [128,768] from PSUM is
~0.66us and the ACT stream is NOT the wall-clock pacer; fp8 DoubleRow
matmuls run at ~1 row/cycle (only the halved pass count helps); DVE psum
copies ~0.54us; InstReciprocal ~4.7us regardless of partition count;
cross-engine chains cost ~1us per semaphore wake, so every per-head
dependency is pipelined at least 2-3 heads deep.

Problem constants are hardcoded per the harness contract.
"""

import numpy as np
import ml_dtypes

B, N, D, OUT, H, DH = 16, 768, 512, 512, 8, 64
NCORES, BPC = 8, 2
KT = D // 128          # 4 contraction tiles of 128
KP = KT // 2           # 2 DoubleRow pair-tiles
FC_KQR = (2 * OUT) // 128   # 8 feature chunks for k+qr
OC = OUT // 128        # 4 output chunks
MC = N // 128          # 6 position chunks
MP = MC // 2           # 3 DoubleRow position pairs
HP = H // 2            # 4 head pairs
NSPLIT = ((0, 512), (512, 256))   # psum free-dim splits (bank aligned)
WSCALE = 16.0          # host-side fp8 weight prescale (avoids subnormals)
ESCALE = 0.125 / (WSCALE * WSCALE)  # exp scale absorbing k,qr prescale

_CACHE = {}


def _build_program(skip_b_kq, skip_b_va, skip_b_g, skip_b_o, reps=1):
    import os
    from contextlib import ExitStack

    DIAG = frozenset(
        x for x in os.environ.get("KDIAG", "").split(",") if x
    )  # timing-only ablations; breaks numerics

    import concourse.bass as bass
    import concourse.mybir as mybir
    import concourse.tile as tile
    from concourse import bacc

    dt = mybir.dt
    f32, bf, f8 = dt.float32, dt.bfloat16, dt.float8e4
    AF = mybir.ActivationFunctionType
    OP = mybir.AluOpType
    DR = mybir.MatmulPerfMode.DoubleRow

    nc = bacc.Bacc("TRN2", target_bir_lowering=False, debug=False)

    # ---- DRAM parameters (per-core shard) -------------------------------
    xT_d = nc.declare_dram_parameter("xT", [2, BPC, KT, 128, N], bf, isOutput=False)
    x8_d = nc.declare_dram_parameter("x8", [2, BPC, KT, 128, N], f8, isOutput=False)
    wkq_d = nc.declare_dram_parameter("wkq", [2, KT, 128, 2 * OUT], f8, isOutput=False)
    wva_d = nc.declare_dram_parameter("wva", [2, KT, 128, OUT], f8, isOutput=False)
    wg_d = nc.declare_dram_parameter("wg", [2, KT, 128, OUT], bf, isOutput=False)
    wo_d = nc.declare_dram_parameter("wo", [2, KT, 128, OUT], bf, isOutput=False)
    bkq_d = nc.declare_dram_parameter("bkq", [2, 128, FC_KQR], f32, isOutput=False)
    bva_d = nc.declare_dram_parameter("bva", [2, 1, OUT], f32, isOutput=False)
    bgc_d = nc.declare_dram_parameter("bgc", [2, 128, OC], f32, isOutput=False)
    bgr_d = nc.declare_dram_parameter("bgr", [2, 1, OUT], f32, isOutput=False)
    bo_d = nc.declare_dram_parameter("bo", [2, 128, OC], f32, isOutput=False)
    rms_d = nc.declare_dram_parameter("rms", [2, BPC, 128, 1], f32, isOutput=False)
    out_d = nc.declare_dram_parameter("out", [2, BPC, OC, 128, N], f32, isOutput=True)

    with ExitStack() as ctx:
        tc = ctx.enter_context(tile.TileContext(nc))

        const = ctx.enter_context(tc.tile_pool(name="const", bufs=1))
        xpool = ctx.enter_context(tc.tile_pool(name="xp", bufs=4))
        x8pool = ctx.enter_context(tc.tile_pool(name="x8p", bufs=4))
        kqrp = ctx.enter_context(tc.tile_pool(name="kqrp", bufs=2))
        vap = ctx.enter_context(tc.tile_pool(name="vap", bufs=2))
        ep = ctx.enter_context(tc.tile_pool(name="ep", bufs=3))
        atp = ctx.enter_context(tc.tile_pool(name="atp", bufs=3))
        smal = ctx.enter_context(tc.tile_pool(name="smal", bufs=4))
        up = ctx.enter_context(tc.tile_pool(name="up", bufs=3))
        rbp = ctx.enter_context(tc.tile_pool(name="rbp", bufs=3))
        # PSUM: 8 banks.  "pss" 2x[128,768] (4 banks) rotate the score
        # chunks PE->ACT; "pso" 1x (2 banks) holds the att-out accumulator;
        # "psx" 1x (2 banks) serves trans/va/proj/gate matmuls.
        psum = ctx.enter_context(tc.tile_pool(name="psum", bufs=1, space="PSUM"))

        # ---- batch-0 activations first ----------------------------------
        # stream 1 first: the first gate (s=0) needs stream 1's mean, so its
        # x load and reduces lead the startup critical path.  x8 loads go on
        # the ACT hwdge queue so they don't queue behind the bf16 loads.
        x_first, x8_first = [None, None], [None, None]
        for s in (1, 0):
            xt = xpool.tile([128, KT, N], bf, name="x", tag="x")
            nc.sync.dma_start(out=xt, in_=xT_d[s, 0].rearrange("t p n -> p t n"))
            x_first[s] = xt
        for s in (0, 1):
            x8 = x8pool.tile([128, KT, N], f8, name="x8", tag="x8")
            nc.scalar.dma_start(out=x8, in_=x8_d[s, 0].rearrange("t p n -> p t n"))
            x8_first[s] = x8

        rms_all = {}
        for bb in range(BPC):
            for s in range(2):
                rt = const.tile([128, 1], f32, name=f"rms{s}_{bb}")
                nc.sync.dma_start(out=rt, in_=rms_d[s, bb])
                rms_all[(s, bb)] = rt

        # ---- load weights / biases once ---------------------------------
        wkq_sb, wva_sb, wg_sb, wo_sb = [], [], [], []
        bkq_sb, bgc_sb, bo_sb, bva_sb, bgr_sb = [], [], [], [], []
        bgcn_sb, bgrn_sb = [], []
        for s in range(2):
            t_kq = const.tile([128, KT, 2 * OUT], f8, name=f"wkq{s}")
            t_va = const.tile([128, KT, OUT], f8, name=f"wva{s}")
            t_g = const.tile([128, KT, OUT], bf, name=f"wg{s}")
            t_o = const.tile([128, KT, OUT], bf, name=f"wo{s}")
            wkq_sb.append(t_kq)
            wva_sb.append(t_va)
            wg_sb.append(t_g)
            wo_sb.append(t_o)
        for s in range(2):
            if not skip_b_kq:
                t = const.tile([128, FC_KQR], f32, name=f"bkq{s}")
                nc.sync.dma_start(out=t, in_=bkq_d[s])
                bkq_sb.append(t)
            else:
                bkq_sb.append(None)
            if not skip_b_g:
                t = const.tile([128, OC], f32, name=f"bgc{s}")
                nc.sync.dma_start(out=t, in_=bgc_d[s])
                bgc_sb.append(t)
                tn = const.tile([128, OC], f32, name=f"bgcn{s}")
                nc.vector.tensor_scalar_mul(tn, t, -1.0)
                bgcn_sb.append(tn)
                t = const.tile([1, OUT], f32, name=f"bgr{s}")
                nc.sync.dma_start(out=t, in_=bgr_d[s])
                bgr_sb.append(t)
                tn = const.tile([1, OUT], f32, name=f"bgrn{s}")
                nc.vector.tensor_scalar_mul(tn, t, -1.0)
                bgrn_sb.append(tn)
            else:
                bgc_sb.append(None)
                bgcn_sb.append(None)
                bgr_sb.append(None)
                bgrn_sb.append(None)
            if not skip_b_o:
                t = const.tile([128, OC], f32, name=f"bo{s}")
                nc.sync.dma_start(out=t, in_=bo_d[s])
                bo_sb.append(t)
            else:
                bo_sb.append(None)
            if not skip_b_va:
                t = const.tile([1, OUT], f32, name=f"bva{s}")
                nc.sync.dma_start(out=t, in_=bva_d[s])
                bva_sb.append(t)
            else:
                bva_sb.append(None)
        ident11 = const.tile([1, 1], f32, name="ident11")
        nc.vector.memset(ident11, 1.0)
        if "nogate" in DIAG:
            const_g2 = const.tile([128, OC], f32, name="cg2")
            nc.vector.memset(const_g2, 1.0)
            const_G = const.tile([128, OUT], bf, name="cG")
            nc.vector.memset(const_G, 1.0)
        nc.gpsimd.dma_start(out=wkq_sb[0], in_=wkq_d[0].rearrange("t p f -> p t f"))
        nc.gpsimd.dma_start(out=wg_sb[0], in_=wg_d[0].rearrange("t p f -> p t f"))
        nc.gpsimd.dma_start(out=wg_sb[1], in_=wg_d[1].rearrange("t p f -> p t f"))
        nc.gpsimd.dma_start(out=wva_sb[0], in_=wva_d[0].rearrange("t p f -> p t f"))
        nc.gpsimd.dma_start(out=wkq_sb[1], in_=wkq_d[1].rearrange("t p f -> p t f"))
        nc.sync.dma_start(out=wva_sb[1], in_=wva_d[1].rearrange("t p f -> p t f"))
        nc.sync.dma_start(out=wo_sb[0], in_=wo_d[0].rearrange("t p f -> p t f"))
        nc.sync.dma_start(out=wo_sb[1], in_=wo_d[1].rearrange("t p f -> p t f"))

        # ---- interleaved per-unit emission ------------------------------

        def gen_prep(rep_i, b, st):
            if rep_i == 0 and b == 0:
                st["x"], st["x8"] = x_first, x8_first
            else:
                st["x"], st["x8"] = [], []
                for s in range(2):
                    xt = xpool.tile([128, KT, N], bf, name="x", tag="x")
                    nc.sync.dma_start(
                        out=xt, in_=xT_d[s, b].rearrange("t p n -> p t n")
                    )
                    st["x"].append(xt)
                for s in range(2):
                    x8 = x8pool.tile([128, KT, N], f8, name="x8", tag="x8")
                    nc.sync.dma_start(
                        out=x8, in_=x8_d[s, b].rearrange("t p n -> p t n")
                    )
                    st["x8"].append(x8)
            yield
            if "nogate" in DIAG:
                yield
                st["gcol"] = [const_g2, const_g2]
                st["G"] = [const_G, const_G]
                return
            x_sb = st["x"]
            mean_sb, rms_sb = {}, {}
            for s in (1, 0):  # stream 1 first: gate s=0 needs mean of 1
                rms_sb[s] = rms_all[(s, b)]
                sums = smal.tile([128, KT], f32, name="sums", tag="sums")
                for kt in range(KT):
                    nc.vector.reduce_sum(
                        out=sums[:, kt : kt + 1],
                        in_=x_sb[s][:, kt, :],
                        axis=mybir.AxisListType.X,
                    )
                mean = smal.tile([128, KT], bf, name="mean", tag="mean")
                nc.vector.tensor_copy(mean, sums)
                mean_sb[s] = mean
            yield
            # row-only gate computation: the per-oc column path (16 tiny PE
            # matmuls + 4 ACT exps + 4 DVE ops per stream) cost ~1us per
            # cross-engine hop on HW.  Compute g and g^2 as rows, then turn
            # g^2 into per-partition columns with one DRAM round trip per
            # batch (latency hidden: prep runs a unit ahead).
            gcol_sb, G_sb = [], []
            for s in range(2):
                o = 1 - s
                # sigmoid via exp (stay in ACT's exp table set): rms_d
                # carries -1/mask_sum, so e = exp(-z) and g = 1 + 1/(1+e)
                pr = psum.tile([1, OUT], f32, name="pr", tag="psx")
                for kt in range(KT):
                    nc.tensor.matmul(
                        pr,
                        lhsT=mean_sb[o][:, kt : kt + 1],
                        rhs=wg_sb[s][:, kt, :],
                        start=(kt == 0),
                        stop=(kt == KT - 1),
                    )
                sig_r = smal.tile([1, OUT], f32, name="sig_r", tag="sig_r", bufs=2)
                if skip_b_g:
                    nc.scalar.activation(
                        out=sig_r, in_=pr, func=AF.Exp, scale=rms_sb[o][0:1, :]
                    )
                else:
                    tmp_r = smal.tile([1, OUT], f32, name="tmp_r", tag="tmp_r", bufs=2)
                    nc.vector.scalar_tensor_tensor(
                        out=tmp_r,
                        in0=pr,
                        scalar=rms_sb[o][0:1, :],
                        in1=bgrn_sb[s],
                        op0=OP.mult,
                        op1=OP.add,
                    )
                    nc.scalar.activation(out=sig_r, in_=tmp_r, func=AF.Exp)
                t1r = smal.tile([1, OUT], f32, name="t1r", tag="t1r", bufs=2)
                nc.vector.tensor_scalar_add(t1r, sig_r, 1.0)
                rr = smal.tile([1, OUT], f32, name="rr", tag="rr", bufs=2)
                nc.vector.reciprocal_approx_fast(out=rr, in_=t1r)
                grow = smal.tile([1, OUT], bf, name="grow", tag="grow", bufs=2)
                nc.vector.tensor_scalar_add(grow, rr, 1.0)
                G = rbp.tile([128, OUT], bf, name="G", tag="G", bufs=2)
                nc.gpsimd.partition_broadcast(G, grow)
                G_sb.append(G)
                g2row = smal.tile([1, OUT], f32, name="g2row", tag="g2row", bufs=2)
                nc.vector.tensor_mul(g2row, grow, grow)
                # turn the g^2 row into per-partition columns with 4 tiny PE
                # transposes (no DMA, no extra engine hops)
                pgt = psum.tile([128, OC], f32, name="pgt", tag="psx")
                for oc in range(OC):
                    nc.tensor.transpose(
                        pgt[:, oc : oc + 1],
                        g2row[0:1, oc * 128 : (oc + 1) * 128],
                        ident11,
                    )
                g2c = smal.tile([128, OC], f32, name="g2c", tag="g2c", bufs=2)
                nc.vector.tensor_copy(g2c, pgt)
                gcol_sb.append(g2c)
                yield
            st["gcol"], st["G"] = gcol_sb, G_sb

        def gen_trans(st, s):
            x8 = st["x8"][s]
            gcol_sb = st["gcol"]
            kqr = kqrp.tile([128, FC_KQR, N], bf, name="kqr", tag="kqr")
            st[("kqr", s)] = kqr
            # k/qr chunk pairs in head-pair order so pair-0 scores can start
            # after the first two chunks.  Psum alternates between the two
            # filler slots so the PE->DVE drain ping-pong pipelines.
            for fi, fc in enumerate((0, OC, 1, OC + 1, 2, OC + 2, 3, OC + 3)):
                pt = psum.tile([128, N], f32, name="pt", tag="psx" if fi % 2 else "pso")
                for i in range(KP):
                    for n0, nw in NSPLIT:
                        nc.tensor.matmul(
                            pt[:, n0 : n0 + nw],
                            lhsT=wkq_sb[s][:, 2 * i : 2 * i + 2, fc * 128 : (fc + 1) * 128],
                            rhs=x8[:, 2 * i : 2 * i + 2, n0 : n0 + nw],
                            start=(i == 0),
                            stop=(i == KP - 1),
                            perf_mode=DR,
                        )
                if fc < OC:
                    gsl = gcol_sb[s][:, fc : fc + 1]
                    if skip_b_kq:
                        nc.vector.tensor_scalar_mul(kqr[:, fc, :], pt, gsl)
                    else:
                        bg2 = smal.tile([128, 1], f32, name="bg2", tag="bg2")
                        nc.vector.tensor_mul(bg2, bkq_sb[s][:, fc : fc + 1], gsl)
                        nc.scalar.activation(
                            out=kqr[:, fc, :],
                            in_=pt,
                            func=AF.Identity,
                            bias=bg2,
                            scale=gsl,
                        )
                else:
                    if skip_b_kq:
                        nc.vector.tensor_copy(kqr[:, fc, :], pt)
                    else:
                        nc.scalar.activation(
                            out=kqr[:, fc, :],
                            in_=pt,
                            func=AF.Identity,
                            bias=bkq_sb[s][:, fc : fc + 1],
                        )
                yield

            va = vap.tile([128, MC, H, DH + 2], f8, name="va", tag="va")
            st[("va", s)] = va
            with nc.allow_low_precision("fp8 attention values"):
                nc.vector.memset(va[:, :, :, DH : DH + 1], WSCALE)
                nc.vector.memset(va[:, :, :, DH + 1 : DH + 2], 0.0)
            G_h = st["G"][s].rearrange("p (h d) -> p h d", h=H)
            for mc in range(MC):
                pv = psum.tile([128, OUT], f32, name="pv", tag="psx" if mc % 2 else "pso")
                for i in range(KP):
                    nc.tensor.matmul(
                        pv,
                        lhsT=x8[:, 2 * i : 2 * i + 2, mc * 128 : (mc + 1) * 128],
                        rhs=wva_sb[s][:, 2 * i : 2 * i + 2, :],
                        start=(i == 0),
                        stop=(i == KP - 1),
                        perf_mode=DR,
                    )
                pv_h = pv.rearrange("p (h d) -> p h d", h=H)
                with nc.allow_low_precision("fp8 attention values"):
                    nc.vector.tensor_mul(va[:, mc, :, 0:DH], pv_h, G_h)
                    if not skip_b_va:
                        bgr_row = smal.tile([1, OUT], f32, name="bgr_row", tag="bgrr")
                        nc.vector.tensor_mul(bgr_row, bva_sb[s], st["G"][s][0:1, :])
                        bg = rbp.tile([128, OUT], f32, name="bg", tag="bg")
                        nc.gpsimd.partition_broadcast(bg, bgr_row)
                        nc.vector.tensor_add(
                            va[:, mc, :, 0:DH],
                            va[:, mc, :, 0:DH],
                            bg.rearrange("p (h d) -> p h d", h=H),
                        )
                yield

        def gen_heads(st, s, last=False):
            xt = st["x"][s]
            kqr = st[("kqr", s)]
            at = atp.tile([128, OC, N], bf, name="at", tag="at")
            st[("at", s)] = at

            def emit_scores(hp, e8):
                # both heads of the pair interleaved at mc granularity:
                # head 2hp on PE rows 0-63, head 2hp+1 on rows 64-127 run
                # concurrently (row tiling).
                for mc in range(MC):
                    for half in range(2):
                        po = 64 * half
                        ps_s = psum.tile([128, N], f32, name="ps_s", tag="pss", bufs=2)
                        lhsT = kqr[po : po + 64, hp, mc * 128 : (mc + 1) * 128]
                        for n0, nw in NSPLIT:
                            nc.tensor.matmul(
                                ps_s[:, n0 : n0 + nw],
                                lhsT=lhsT,
                                rhs=kqr[po : po + 64, OC + hp, n0 : n0 + nw],
                                start=True,
                                stop=True,
                            )
                        with nc.allow_low_precision("fp8 attention probs"):
                            if "tinyexp" in DIAG:
                                nc.scalar.activation(
                                    out=e8[:, half, mc, 0:32],
                                    in_=ps_s[:, 0:32],
                                    func=AF.Exp,
                                    scale=ESCALE,
                                )
                            else:
                                nc.scalar.activation(
                                    out=e8[:, half, mc, :],
                                    in_=ps_s,
                                    func=AF.Exp,
                                    scale=ESCALE,
                                )
                    yield

            pending_norm = []  # (hp, po, o_sb, rb) with broadcast in flight

            def emit_norm():
                # at-mul + residual for the oldest pending head; its rb
                # broadcast has been in flight while the next head's att
                # matmuls ran, so DVE never waits on the DMA round trip.
                # residual add on DVE (all-SBUF bf16): Pool must stay
                # pbcast-only — mixing gpsimd op families reloads the Q7
                # overlay per switch, serializing the whole norm chain
                nhp, npo, no_sb, nrb = pending_norm.pop(0)
                if nrb is None:  # KDIAG=nonorm: same DVE volume, no rb dep
                    nc.vector.tensor_copy(
                        at[npo : npo + 64, nhp, :], no_sb[0:DH, :]
                    )
                else:
                    nc.vector.tensor_mul(
                        at[npo : npo + 64, nhp, :], no_sb[0:DH, :], nrb
                    )
                nc.vector.tensor_add(
                    at[npo : npo + 64, nhp, :],
                    at[npo : npo + 64, nhp, :],
                    xt[npo : npo + 64, nhp, :],
                )

            def emit_att(hp, e8):
                va = st[("va", s)]  # created by gen_trans's va section
                if "noatt" in DIAG:
                    for half in range(2):
                        yield
                    return
                for half in range(2):
                    h = 2 * hp + half
                    po = 64 * half
                    po_t = psum.tile([DH + 2, N], f32, name="po_t", tag="pso")
                    for n0, nw in NSPLIT:
                        for i in range(MP):
                            nc.tensor.matmul(
                                po_t[:, n0 : n0 + nw],
                                lhsT=va[:, 2 * i : 2 * i + 2, h, :],
                                rhs=e8[:, half, 2 * i : 2 * i + 2, n0 : n0 + nw],
                                start=(i == 0),
                                stop=(i == MP - 1),
                                perf_mode=DR,
                            )
                    o_sb = rbp.tile([DH + 1, N], bf, name="o_sb", tag="o_sb", bufs=4)
                    nc.vector.tensor_copy(o_sb, po_t[0 : DH + 1, :])
                    if "nonorm" in DIAG:
                        pending_norm.append((hp, po, o_sb, None))
                        if len(pending_norm) > 1:
                            emit_norm()
                        yield
                        continue
                    # InstReciprocal costs ~4.7us on HW; approx_fast (~18
                    # correct bits, one custom-DVE op) is ~5x cheaper and far
                    # exceeds the bf16 precision of the multiply it feeds.
                    # Its fp32 bit-trick seed needs a partition-0 SBUF input:
                    # stage the PSUM denominator row first.
                    den32 = smal.tile([1, N], f32, name="den32", tag="den32", bufs=2)
                    nc.vector.tensor_copy(den32, po_t[DH : DH + 1, :])
                    r_row = smal.tile([1, N], f32, name="r_row", tag="r_row", bufs=3)
                    nc.vector.reciprocal_approx_fast(out=r_row, in_=den32)
                    # broadcast 1/den across 64 partitions on the Pool engine:
                    # no DMA round trip, keeps SP free for bulk loads
                    rb = rbp.tile([64, N], f32, name="rb", tag="rb", bufs=4)
                    nc.gpsimd.partition_broadcast(rb, r_row)
                    # depth-3 norm pipeline: the DVE->Pool->DVE round trip
                    # (recip -> pbcast -> norm-mul) costs ~3.5us in sem-wake
                    # latency on HW; emitting the norm 3 half-heads late keeps
                    # DVE from ever blocking on rb
                    pending_norm.append((hp, po, o_sb, rb))
                    if len(pending_norm) > 3:
                        emit_norm()
                    yield

            # software-pipelined: scores of pair hp, then att of pair hp-1.
            # For the final unit there is no later exp work to hide under, so
            # run att eagerly right after each pair's scores to shorten the
            # drain tail.
            if last:
                for hp in range(HP):
                    e8 = ep.tile([128, 2, MC, N], f8, name="e", tag="e")
                    yield from emit_scores(hp, e8)
                    yield from emit_att(hp, e8)
            else:
                prev_e8 = None
                for hp in range(HP):
                    e8 = ep.tile([128, 2, MC, N], f8, name="e", tag="e")
                    yield from emit_scores(hp, e8)
                    if prev_e8 is not None:
                        yield from emit_att(hp - 1, prev_e8)
                    prev_e8 = e8
                yield from emit_att(HP - 1, prev_e8)
            while pending_norm:
                emit_norm()

        def gen_proj(st, s, b):
            at = st[("at", s)]
            if "noproj" in DIAG or "noatt" in DIAG:
                for oc in range(OC):
                    yield
                return
            for oc in range(OC):
                pu = psum.tile([128, N], f32, name="pu", tag="psx" if oc % 2 else "pso")
                for kt in range(KT):
                    for n0, nw in NSPLIT:
                        nc.tensor.matmul(
                            pu[:, n0 : n0 + nw],
                            lhsT=wo_sb[s][:, kt, oc * 128 : (oc + 1) * 128],
                            rhs=at[:, kt, n0 : n0 + nw],
                            start=(kt == 0),
                            stop=(kt == KT - 1),
                        )
                u_sb = up.tile([128, N], f32, name="u", tag="u")
                if skip_b_o:
                    nc.vector.tensor_copy(u_sb, pu)
                else:
                    nc.vector.tensor_scalar_add(u_sb, pu, bo_sb[s][:, oc : oc + 1])
                nc.sync.dma_start(out=out_d[s, b, oc], in_=u_sb)
                yield

        def drain(g):
            if g is not None:
                for _ in g:
                    pass

        units = [(r, bb, s) for r in range(reps) for bb in range(BPC) for s in range(2)]
        states = {}

        def state_for(r, bb):
            return states.setdefault((r, bb), {})

        from itertools import islice

        st0 = state_for(units[0][0], units[0][1])
        drain(gen_prep(units[0][0], units[0][1], st0))
        # stream unit-0's trans: emit the first two k/qr chunks (pair-0
        # scores inputs), leave the rest as a filler inside its head loop
        tr0 = gen_trans(st0, units[0][2])
        for _ in islice(tr0, 2):
            pass

        pending_proj = None
        pending_heads = {}  # unit index -> (generator, yields already consumed)
        for i, (r, bb, s) in enumerate(units):
            st = state_for(r, bb)
            fillers = []
            if i == 0:
                fillers.append(tr0)
            if pending_proj is not None:
                fillers.append(pending_proj)
            nxt_heads = None
            pre = [0]
            if i + 1 < len(units):
                rn, bn, sn = units[i + 1]
                stn = state_for(rn, bn)
                if (rn, bn) != (r, bb):
                    fillers.append(gen_prep(rn, bn, stn))
                fillers.append(gen_trans(stn, sn))
                # cross-unit head overlap: after the next unit's trans/va
                # fillers drain, let its first score chunks emit inside THIS
                # unit's head loop so ACT's exp stream never drains at the
                # unit boundary

                def counted(g, cnt):
                    for x in g:
                        cnt[0] += 1
                        yield x

                nxt_heads = gen_heads(stn, sn, last=(i + 1 == len(units) - 1))
                fillers.append(islice(counted(nxt_heads, pre), 16))
            heads, done = pending_heads.pop(i, (None, 0))
            if heads is None:
                heads = gen_heads(st, s)
            nyield = HP * MC + H  # score-chunk yields + att-half yields
            for h in range(nyield - done):
                if next(heads, StopIteration) is StopIteration:
                    break
                # drip-feed fillers: 1/yield early so serialized trans/proj
                # drains never dam up the PE stream ahead of score matmuls,
                # 2/yield later to finish the supply before this unit ends
                for _ in range(1 if h < 12 else 2):
                    while fillers:
                        try:
                            next(fillers[0])
                            break
                        except StopIteration:
                            fillers.pop(0)
                    else:
                        break
            drain(heads)
            for g in fillers:
                drain(g)
            if nxt_heads is not None:
                pending_heads[i + 1] = (nxt_heads, pre[0])
            pending_proj = gen_proj(st, s, bb)
        drain(pending_proj)

    nc.finalize()
    return nc


def _prep_inputs(inputs):
    bf16 = ml_dtypes.bfloat16
    fp8 = ml_dtypes.float8_e4m3
    f32 = np.float32

    def arr(name):
        return np.asarray(inputs[name], f32)

    v, q = arr("v"), arr("q")
    v_mask, q_mask = arr("v_mask"), arr("q_mask")

    def prep_x(x, dtype):  # [B, N, D] -> [B, KT, 128, N] (transposed)
        xt = np.ascontiguousarray(x.transpose(0, 2, 1))
        return xt.reshape(B, KT, 128, N).astype(dtype)

    def prep_w(w, dtype, scale=1.0):  # [F, D] -> [KT, 128, F]  (= w.T tiled)
        wt = np.ascontiguousarray(w.T) * f32(scale)
        return wt.reshape(KT, 128, -1).astype(dtype)

    def col128(bias, scale=1.0):  # [F] -> [128, F//128] f32 columns
        return np.ascontiguousarray((bias * f32(scale)).reshape(-1, 128).T).astype(f32)

    w_v, w_q = arr("w_v"), arr("w_q")
    b_v, b_q = arr("b_v"), arr("b_q")
    w_q4v, w_v4q = arr("w_q4v"), arr("w_v4q")
    b_q4v, b_v4q = arr("b_q4v"), arr("b_v4q")
    w_vo, w_qo = arr("w_vo"), arr("w_qo")
    b_vo, b_qo = arr("b_vo"), arr("b_qo")

    xT = np.stack([prep_x(v, bf16), prep_x(q, bf16)])  # [2, B, KT, 128, N]
    x8 = np.stack([prep_x(v, fp8), prep_x(q, fp8)])
    wkq = np.stack(
        [prep_w(w_v[: 2 * OUT], fp8, WSCALE), prep_w(w_q[: 2 * OUT], fp8, WSCALE)]
    )
    wva = np.stack(
        [prep_w(w_v[2 * OUT :], fp8, WSCALE), prep_w(w_q[2 * OUT :], fp8, WSCALE)]
    )
    wg = np.stack([prep_w(w_q4v, bf16), prep_w(w_v4q, bf16)])  # stream 0 gated via q_mean
    wo = np.stack([prep_w(w_vo, bf16), prep_w(w_qo, bf16)])
    bkq = np.stack(
        [col128(b_v[: 2 * OUT], WSCALE), col128(b_q[: 2 * OUT], WSCALE)]
    )
    bva = np.stack(
        [b_v[2 * OUT :][None, :], b_q[2 * OUT :][None, :]]
    ).astype(f32) * f32(WSCALE)
    bgc = np.stack([col128(b_q4v), col128(b_v4q)])
    bgr = np.stack([b_q4v[None, :], b_v4q[None, :]]).astype(f32)
    bo = np.stack([col128(b_vo), col128(b_qo)])

    rms_v = -1.0 / v_mask.sum(1)  # [B]; negative: kernel computes exp(-z)
    rms_q = -1.0 / q_mask.sum(1)
    rms = np.empty((2, B, 128, 1), f32)
    rms[0] = np.broadcast_to(rms_v[:, None, None], (B, 128, 1))
    rms[1] = np.broadcast_to(rms_q[:, None, None], (B, 128, 1))

    skips = (
        bool((b_v[: 2 * OUT] == 0).all() and (b_q[: 2 * OUT] == 0).all()),
        bool((b_v[2 * OUT :] == 0).all() and (b_q[2 * OUT :] == 0).all()),
        bool((b_q4v == 0).all() and (b_v4q == 0).all()),
        bool((b_vo == 0).all() and (b_qo == 0).all()),
    )

    in_maps = []
    for c in range(NCORES):
        sl = slice(c * BPC, (c + 1) * BPC)
        in_maps.append(
            {
                "xT": np.ascontiguousarray(xT[:, sl]),
                "x8": np.ascontiguousarray(x8[:, sl]),
                "wkq": wkq,
                "wva": wva,
                "wg": wg,
                "wo": wo,
                "bkq": bkq,
                "bva": bva,
                "bgc": bgc,
                "bgr": bgr,
                "bo": bo,
                "rms": np.ascontiguousarray(rms[:, sl]),
            }
        )
    return in_maps, skips


def _get_program(skips, reps=1):
    import os

    key = ("prog", skips, reps, os.environ.get("KDIAG", ""))
    if key not in _CACHE:
        _CACHE[key] = _build_program(*skips, reps=reps)
    return _CACHE[key]


def kernel(trace=False, **inputs):
    from concourse.bass_utils import run_bass_kernel_spmd

    in_maps, skips = _prep_inputs(inputs)
    nc = _get_program(skips)
    res = run_bass_kernel_spmd(
        nc, in_maps, core_ids=list(range(NCORES)), trace=trace
    )
    _CACHE["last_result"] = res
    outs = np.stack([r["out"] for r in res.results])  # [8, 2, BPC, OC, 128, N]
    u = outs.reshape(NCORES, 2, BPC, D, N)
    uv = u[:, 0].reshape(B, D, N).transpose(0, 2, 1)
    uq = u[:, 1].reshape(B, D, N).transpose(0, 2, 1)
    return (
        np.ascontiguousarray(uv).astype(np.float32),
        np.ascontiguousarray(uq).astype(np.float32),
    )



# revision 29
# speedup vs baseline: 1.2133x; 1.0536x over previous
"""Trainium2 Bass kernel for DyIntraModalityUpdate (dual gated self-attention).

Strategy
--------
Data-parallel over batch: 16 batches -> 8 NeuronCores x 2 batches, zero
collectives.  Each core processes 4 independent "units" (2 batches x
{v-stream, q-stream}); the only cross-stream coupling is the gates
(v_mean gates q's attention and vice versa), computed per batch before the
per-stream work.

All heavy compute is done in a transposed layout [feature, position]:
  - k/qr projections are computed directly transposed: kqrT[f, r], via
    fp8e4m3 DoubleRow matmuls (2 k-tiles of 128 contracted per instruction,
    ~1.5-2x PE throughput).  Weights are pre-scaled x16 on the host so fp8
    values avoid the subnormal range; the resulting 256x score scale is
    folded into the softmax exp scale.
  - scores: per head pair (2k, 2k+1) the S^T matmuls are emitted
    interleaved; the two heads' lhsT/rhs live at partitions 0-63 / 64-127,
    so the PE row-tiling (tile_position rows 0 and 64) runs both heads'
    matmuls concurrently (~2x).
  - E^T = exp(S^T * 0.125/256) written as fp8e4m3 (attention here is very
    flat - probs ~ 1/768 - so fp8 quantization noise averages out).
  - va is computed in natural layout [position, feature] (fp8 DoubleRow),
    gated, stored as fp8 with a 16.0-column appended so the att-out
    matmul's extra output row yields 16x the softmax denominator
    (compensating the x16 va scale exactly after the reciprocal).
  - att-out O^T = va_ext^T @ E^T via fp8 DoubleRow over position-tile
    pairs.
  - normalization multiplies O^T rows by 1/denominator: the denominator row
    is staged to SBUF f32, inverted with reciprocal_approx_fast (the plain
    InstReciprocal costs ~4.7us on HW), broadcast across 64 partitions with
    gpsimd.partition_broadcast (no DMA round trip), and applied 3 half-heads
    later (depth-3 norm pipeline) so DVE never blocks on the Pool hop.
  - residual add on DVE; the Pool engine runs ONLY partition_broadcast:
    mixing gpsimd op families reloads the Q7 overlay (~8us per switch).
  - final projection stays bf16 (precision: the residual feeds the output
    directly).

HW notes (measured, CoreSim's model differs): exp [128,768] from PSUM is
~0.66us and the ACT stream is NOT the wall-clock pacer; fp8 DoubleRow
matmuls run at ~1 row/cycle (only the halved pass count helps); DVE psum
copies ~0.54us; InstReciprocal ~4.7us regardless of partition count;
cross-engine chains cost ~1us per semaphore wake, so every per-head
dependency is pipelined at least 2-3 heads deep.

Problem constants are hardcoded per the harness contract.
"""

import numpy as np
import ml_dtypes

B, N, D, OUT, H, DH = 16, 768, 512, 512, 8, 64
NCORES, BPC = 8, 2
KT = D // 128          # 4 contraction tiles of 128
KP = KT // 2           # 2 DoubleRow pair-tiles
FC_KQR = (2 * OUT) // 128   # 8 feature chunks for k+qr
OC = OUT // 128        # 4 output chunks
MC = N // 128          # 6 position chunks
MP = MC // 2           # 3 DoubleRow position pairs
HP = H // 2            # 4 head pairs
NSPLIT = ((0, 512), (512, 256))   # psum free-dim splits (bank aligned)
WSCALE = 16.0          # host-side fp8 weight prescale (avoids subnormals)
ESCALE = 0.125 / (WSCALE * WSCALE)  # exp scale absorbing k,qr prescale

_CACHE = {}


def _build_program(skip_b_kq, skip_b_va, skip_b_g, skip_b_o, reps=1):
    import os
    from contextlib import ExitStack

    DIAG = frozenset(
        x for x in os.environ.get("KDIAG", "").split(",") if x
    )  # timing-only ablations; breaks numerics

    import concourse.bass as bass
    import concourse.mybir as mybir
    import concourse.tile as tile
    from concourse import bacc

    dt = mybir.dt
    f32, bf, f8 = dt.float32, dt.bfloat16, dt.float8e4
    AF = mybir.ActivationFunctionType
    OP = mybir.AluOpType
    DR = mybir.MatmulPerfMode.DoubleRow

    nc = bacc.Bacc("TRN2", target_bir_lowering=False, debug=False)

    # ---- DRAM parameters (per-core shard) -------------------------------
    xT_d = nc.declare_dram_parameter("xT", [2, BPC, KT, 128, N], bf, isOutput=False)
    x8_d = nc.declare_dram_parameter("x8", [2, BPC, KT, 128, N], f8, isOutput=False)
    wkq_d = nc.declare_dram_parameter("wkq", [2, KT, 128, 2 * OUT], f8, isOutput=False)
    wva_d = nc.declare_dram_parameter("wva", [2, KT, 128, OUT], f8, isOutput=False)
    wg_d = nc.declare_dram_parameter("wg", [2, KT, 128, OUT], bf, isOutput=False)
    wo_d = nc.declare_dram_parameter("wo", [2, KT, 128, OUT], bf, isOutput=False)
    bkq_d = nc.declare_dram_parameter("bkq", [2, 128, FC_KQR], f32, isOutput=False)
    bva_d = nc.declare_dram_parameter("bva", [2, 1, OUT], f32, isOutput=False)
    bgc_d = nc.declare_dram_parameter("bgc", [2, 128, OC], f32, isOutput=False)
    bgr_d = nc.declare_dram_parameter("bgr", [2, 1, OUT], f32, isOutput=False)
    bo_d = nc.declare_dram_parameter("bo", [2, 128, OC], f32, isOutput=False)
    rms_d = nc.declare_dram_parameter("rms", [2, BPC, 128, 1], f32, isOutput=False)
    out_d = nc.declare_dram_parameter("out", [2, BPC, OC, 128, N], f32, isOutput=True)

    with ExitStack() as ctx:
        tc = ctx.enter_context(tile.TileContext(nc))

        const = ctx.enter_context(tc.tile_pool(name="const", bufs=1))
        xpool = ctx.enter_context(tc.tile_pool(name="xp", bufs=4))
        x8pool = ctx.enter_context(tc.tile_pool(name="x8p", bufs=4))
        kqrp = ctx.enter_context(tc.tile_pool(name="kqrp", bufs=2))
        vap = ctx.enter_context(tc.tile_pool(name="vap", bufs=2))
        ep = ctx.enter_context(tc.tile_pool(name="ep", bufs=3))
        atp = ctx.enter_context(tc.tile_pool(name="atp", bufs=3))
        smal = ctx.enter_context(tc.tile_pool(name="smal", bufs=4))
        up = ctx.enter_context(tc.tile_pool(name="up", bufs=3))
        rbp = ctx.enter_context(tc.tile_pool(name="rbp", bufs=3))
        # PSUM: 8 banks.  "pss" 2x[128,768] (4 banks) rotate the score
        # chunks PE->ACT; "pso" 1x (2 banks) holds the att-out accumulator;
        # "psx" 1x (2 banks) serves trans/va/proj/gate matmuls.
        psum = ctx.enter_context(tc.tile_pool(name="psum", bufs=1, space="PSUM"))

        # ---- batch-0 activations first ----------------------------------
        # stream 1 first: the first gate (s=0) needs stream 1's mean, so its
        # x load and reduces lead the startup critical path.  x8 loads go on
        # the ACT hwdge queue so they don't queue behind the bf16 loads.
        x_first, x8_first = [None, None], [None, None]
        for s in (1, 0):
            xt = xpool.tile([128, KT, N], bf, name="x", tag="x")
            nc.sync.dma_start(out=xt, in_=xT_d[s, 0].rearrange("t p n -> p t n"))
            x_first[s] = xt
        for s in (0, 1):
            x8 = x8pool.tile([128, KT, N], f8, name="x8", tag="x8")
            nc.scalar.dma_start(out=x8, in_=x8_d[s, 0].rearrange("t p n -> p t n"))
            x8_first[s] = x8

        rms_all = {}
        for bb in range(BPC):
            for s in range(2):
                rt = const.tile([128, 1], f32, name=f"rms{s}_{bb}")
                nc.sync.dma_start(out=rt, in_=rms_d[s, bb])
                rms_all[(s, bb)] = rt

        # ---- load weights / biases once ---------------------------------
        wkq_sb, wva_sb, wg_sb, wo_sb = [], [], [], []
        bkq_sb, bgc_sb, bo_sb, bva_sb, bgr_sb = [], [], [], [], []
        bgcn_sb, bgrn_sb = [], []
        for s in range(2):
            t_kq = const.tile([128, KT, 2 * OUT], f8, name=f"wkq{s}")
            t_va = const.tile([128, KT, OUT], f8, name=f"wva{s}")
            t_g = const.tile([128, KT, OUT], bf, name=f"wg{s}")
            t_o = const.tile([128, KT, OUT], bf, name=f"wo{s}")
            wkq_sb.append(t_kq)
            wva_sb.append(t_va)
            wg_sb.append(t_g)
            wo_sb.append(t_o)
        for s in range(2):
            if not skip_b_kq:
                t = const.tile([128, FC_KQR], f32, name=f"bkq{s}")
                nc.sync.dma_start(out=t, in_=bkq_d[s])
                bkq_sb.append(t)
            else:
                bkq_sb.append(None)
            if not skip_b_g:
                t = const.tile([128, OC], f32, name=f"bgc{s}")
                nc.sync.dma_start(out=t, in_=bgc_d[s])
                bgc_sb.append(t)
                tn = const.tile([128, OC], f32, name=f"bgcn{s}")
                nc.vector.tensor_scalar_mul(tn, t, -1.0)
                bgcn_sb.append(tn)
                t = const.tile([1, OUT], f32, name=f"bgr{s}")
                nc.sync.dma_start(out=t, in_=bgr_d[s])
                bgr_sb.append(t)
                tn = const.tile([1, OUT], f32, name=f"bgrn{s}")
                nc.vector.tensor_scalar_mul(tn, t, -1.0)
                bgrn_sb.append(tn)
            else:
                bgc_sb.append(None)
                bgcn_sb.append(None)
                bgr_sb.append(None)
                bgrn_sb.append(None)
            if not skip_b_o:
                t = const.tile([128, OC], f32, name=f"bo{s}")
                nc.sync.dma_start(out=t, in_=bo_d[s])
                bo_sb.append(t)
            else:
                bo_sb.append(None)
            if not skip_b_va:
                t = const.tile([1, OUT], f32, name=f"bva{s}")
                nc.sync.dma_start(out=t, in_=bva_d[s])
                bva_sb.append(t)
            else:
                bva_sb.append(None)
        ident11 = const.tile([1, 1], f32, name="ident11")
        nc.vector.memset(ident11, 1.0)
        if "nogate" in DIAG:
            const_g2 = const.tile([128, OC], f32, name="cg2")
            nc.vector.memset(const_g2, 1.0)
            const_G = const.tile([128, OUT], bf, name="cG")
            nc.vector.memset(const_G, 1.0)
        nc.gpsimd.dma_start(out=wkq_sb[0], in_=wkq_d[0].rearrange("t p f -> p t f"))
        nc.gpsimd.dma_start(out=wg_sb[0], in_=wg_d[0].rearrange("t p f -> p t f"))
        nc.gpsimd.dma_start(out=wg_sb[1], in_=wg_d[1].rearrange("t p f -> p t f"))
        nc.gpsimd.dma_start(out=wva_sb[0], in_=wva_d[0].rearrange("t p f -> p t f"))
        nc.gpsimd.dma_start(out=wkq_sb[1], in_=wkq_d[1].rearrange("t p f -> p t f"))
        nc.sync.dma_start(out=wva_sb[1], in_=wva_d[1].rearrange("t p f -> p t f"))
        nc.sync.dma_start(out=wo_sb[0], in_=wo_d[0].rearrange("t p f -> p t f"))
        nc.sync.dma_start(out=wo_sb[1], in_=wo_d[1].rearrange("t p f -> p t f"))

        # ---- interleaved per-unit emission ------------------------------

        def gen_prep(rep_i, b, st):
            if rep_i == 0 and b == 0:
                st["x"], st["x8"] = x_first, x8_first
            else:
                st["x"], st["x8"] = [], []
                # steady-state x loads go on the ACT hwdge queue: the SP
                # queue carries the out stores, which would delay the means
                # -> gates chain that the next unit's k-gating waits on
                for s in range(2):
                    xt = xpool.tile([128, KT, N], bf, name="x", tag="x")
                    nc.scalar.dma_start(
                        out=xt, in_=xT_d[s, b].rearrange("t p n -> p t n")
                    )
                    st["x"].append(xt)
                for s in range(2):
                    x8 = x8pool.tile([128, KT, N], f8, name="x8", tag="x8")
                    nc.scalar.dma_start(
                        out=x8, in_=x8_d[s, b].rearrange("t p n -> p t n")
                    )
                    st["x8"].append(x8)
            yield
            if "nogate" in DIAG:
                yield
                st["gcol"] = [const_g2, const_g2]
                st["G"] = [const_G, const_G]
                return
            x_sb = st["x"]
            mean_sb, rms_sb = {}, {}
            for s in (1, 0):  # stream 1 first: gate s=0 needs mean of 1
                rms_sb[s] = rms_all[(s, b)]
                sums = smal.tile([128, KT], f32, name="sums", tag="sums")
                for kt in range(KT):
                    nc.vector.reduce_sum(
                        out=sums[:, kt : kt + 1],
                        in_=x_sb[s][:, kt, :],
                        axis=mybir.AxisListType.X,
                    )
                mean = smal.tile([128, KT], bf, name="mean", tag="mean")
                nc.vector.tensor_copy(mean, sums)
                mean_sb[s] = mean
            yield
            # row-only gate computation: the per-oc column path (16 tiny PE
            # matmuls + 4 ACT exps + 4 DVE ops per stream) cost ~1us per
            # cross-engine hop on HW.  Compute g and g^2 as rows, then turn
            # g^2 into per-partition columns with one DRAM round trip per
            # batch (latency hidden: prep runs a unit ahead).
            gcol_sb, G_sb = [], []
            for s in range(2):
                o = 1 - s
                # sigmoid via exp (stay in ACT's exp table set): rms_d
                # carries -1/mask_sum, so e = exp(-z) and g = 1 + 1/(1+e)
                pr = psum.tile([1, OUT], f32, name="pr", tag="psx")
                for kt in range(KT):
                    nc.tensor.matmul(
                        pr,
                        lhsT=mean_sb[o][:, kt : kt + 1],
                        rhs=wg_sb[s][:, kt, :],
                        start=(kt == 0),
                        stop=(kt == KT - 1),
                    )
                sig_r = smal.tile([1, OUT], f32, name="sig_r", tag="sig_r", bufs=2)
                if skip_b_g:
                    nc.scalar.activation(
                        out=sig_r, in_=pr, func=AF.Exp, scale=rms_sb[o][0:1, :]
                    )
                else:
                    tmp_r = smal.tile([1, OUT], f32, name="tmp_r", tag="tmp_r", bufs=2)
                    nc.vector.scalar_tensor_tensor(
                        out=tmp_r,
                        in0=pr,
                        scalar=rms_sb[o][0:1, :],
                        in1=bgrn_sb[s],
                        op0=OP.mult,
                        op1=OP.add,
                    )
                    nc.scalar.activation(out=sig_r, in_=tmp_r, func=AF.Exp)
                t1r = smal.tile([1, OUT], f32, name="t1r", tag="t1r", bufs=2)
                nc.vector.tensor_scalar_add(t1r, sig_r, 1.0)
                rr = smal.tile([1, OUT], f32, name="rr", tag="rr", bufs=2)
                nc.vector.reciprocal_approx_fast(out=rr, in_=t1r)
                grow = smal.tile([1, OUT], bf, name="grow", tag="grow", bufs=2)
                nc.vector.tensor_scalar_add(grow, rr, 1.0)
                G = rbp.tile([128, OUT], bf, name="G", tag="G", bufs=2)
                nc.gpsimd.partition_broadcast(G, grow)
                G_sb.append(G)
                g2row = smal.tile([1, OUT], f32, name="g2row", tag="g2row", bufs=2)
                nc.vector.tensor_mul(g2row, grow, grow)
                # turn the g^2 row into per-partition columns with 4 tiny PE
                # transposes (no DMA, no extra engine hops)
                pgt = psum.tile([128, OC], f32, name="pgt", tag="psx")
                for oc in range(OC):
                    nc.tensor.transpose(
                        pgt[:, oc : oc + 1],
                        g2row[0:1, oc * 128 : (oc + 1) * 128],
                        ident11,
                    )
                g2c = smal.tile([128, OC], f32, name="g2c", tag="g2c", bufs=2)
                nc.vector.tensor_copy(g2c, pgt)
                gcol_sb.append(g2c)
                yield
            st["gcol"], st["G"] = gcol_sb, G_sb

        def gen_trans(st, s):
            x8 = st["x8"][s]
            gcol_sb = st["gcol"]
            kqr = kqrp.tile([128, FC_KQR, N], bf, name="kqr", tag="kqr")
            st[("kqr", s)] = kqr
            # ungated qr chunks (fc >= OC) first: their drains don't wait on
            # the gate chain, giving it ~4 more yield-slots to resolve before
            # a gated k-chunk drain can head-of-line-block the DVE queue.
            for fi, fc in enumerate((OC, OC + 1, 0, 1, OC + 2, 2, OC + 3, 3)):
                pt = psum.tile([128, N], f32, name="pt", tag="psx" if fi % 2 else "pso")
                for i in range(KP):
                    for n0, nw in NSPLIT:
                        nc.tensor.matmul(
                            pt[:, n0 : n0 + nw],
                            lhsT=wkq_sb[s][:, 2 * i : 2 * i + 2, fc * 128 : (fc + 1) * 128],
                            rhs=x8[:, 2 * i : 2 * i + 2, n0 : n0 + nw],
                            start=(i == 0),
                            stop=(i == KP - 1),
                            perf_mode=DR,
                        )
                if fc < OC:
                    gsl = gcol_sb[s][:, fc : fc + 1]
                    if skip_b_kq:
                        nc.vector.tensor_scalar_mul(kqr[:, fc, :], pt, gsl)
                    else:
                        bg2 = smal.tile([128, 1], f32, name="bg2", tag="bg2")
                        nc.vector.tensor_mul(bg2, bkq_sb[s][:, fc : fc + 1], gsl)
                        nc.scalar.activation(
                            out=kqr[:, fc, :],
                            in_=pt,
                            func=AF.Identity,
                            bias=bg2,
                            scale=gsl,
                        )
                else:
                    if skip_b_kq:
                        nc.vector.tensor_copy(kqr[:, fc, :], pt)
                    else:
                        nc.scalar.activation(
                            out=kqr[:, fc, :],
                            in_=pt,
                            func=AF.Identity,
                            bias=bkq_sb[s][:, fc : fc + 1],
                        )
                yield

            va = vap.tile([128, MC, H, DH + 2], f8, name="va", tag="va")
            st[("va", s)] = va
            with nc.allow_low_precision("fp8 attention values"):
                nc.vector.memset(va[:, :, :, DH : DH + 1], WSCALE)
                nc.vector.memset(va[:, :, :, DH + 1 : DH + 2], 0.0)
            G_h = st["G"][s].rearrange("p (h d) -> p h d", h=H)
            for mc in range(MC):
                pv = psum.tile([128, OUT], f32, name="pv", tag="psx" if mc % 2 else "pso")
                for i in range(KP):
                    nc.tensor.matmul(
                        pv,
                        lhsT=x8[:, 2 * i : 2 * i + 2, mc * 128 : (mc + 1) * 128],
                        rhs=wva_sb[s][:, 2 * i : 2 * i + 2, :],
                        start=(i == 0),
                        stop=(i == KP - 1),
                        perf_mode=DR,
                    )
                pv_h = pv.rearrange("p (h d) -> p h d", h=H)
                with nc.allow_low_precision("fp8 attention values"):
                    nc.vector.tensor_mul(va[:, mc, :, 0:DH], pv_h, G_h)
                    if not skip_b_va:
                        bgr_row = smal.tile([1, OUT], f32, name="bgr_row", tag="bgrr")
                        nc.vector.tensor_mul(bgr_row, bva_sb[s], st["G"][s][0:1, :])
                        bg = rbp.tile([128, OUT], f32, name="bg", tag="bg")
                        nc.gpsimd.partition_broadcast(bg, bgr_row)
                        nc.vector.tensor_add(
                            va[:, mc, :, 0:DH],
                            va[:, mc, :, 0:DH],
                            bg.rearrange("p (h d) -> p h d", h=H),
                        )
                yield

        def gen_heads(st, s, last=False):
            xt = st["x"][s]
            kqr = st[("kqr", s)]
            at = atp.tile([128, OC, N], bf, name="at", tag="at")
            st[("at", s)] = at

            def emit_scores(hp, e8):
                # both heads of the pair interleaved at mc granularity:
                # head 2hp on PE rows 0-63, head 2hp+1 on rows 64-127 run
                # concurrently (row tiling).
                for mc in range(MC):
                    for half in range(2):
                        po = 64 * half
                        ps_s = psum.tile([128, N], f32, name="ps_s", tag="pss", bufs=2)
                        lhsT = kqr[po : po + 64, hp, mc * 128 : (mc + 1) * 128]
                        for n0, nw in NSPLIT:
                            nc.tensor.matmul(
                                ps_s[:, n0 : n0 + nw],
                                lhsT=lhsT,
                                rhs=kqr[po : po + 64, OC + hp, n0 : n0 + nw],
                                start=True,
                                stop=True,
                            )
                        with nc.allow_low_precision("fp8 attention probs"):
                            if "tinyexp" in DIAG:
                                nc.scalar.activation(
                                    out=e8[:, half, mc, 0:32],
                                    in_=ps_s[:, 0:32],
                                    func=AF.Exp,
                                    scale=ESCALE,
                                )
                            else:
                                nc.scalar.activation(
                                    out=e8[:, half, mc, :],
                                    in_=ps_s,
                                    func=AF.Exp,
                                    scale=ESCALE,
                                )
                    yield

            pending_norm = []  # (hp, po, o_sb, rb) with broadcast in flight

            def emit_norm():
                # at-mul + residual for the oldest pending head; its rb
                # broadcast has been in flight while the next head's att
                # matmuls ran, so DVE never waits on the DMA round trip.
                # residual add on DVE (all-SBUF bf16): Pool must stay
                # pbcast-only — mixing gpsimd op families reloads the Q7
                # overlay per switch, serializing the whole norm chain
                nhp, npo, no_sb, nrb = pending_norm.pop(0)
                if nrb is None:  # KDIAG=nonorm: same DVE volume, no rb dep
                    nc.vector.tensor_copy(
                        at[npo : npo + 64, nhp, :], no_sb[0:DH, :]
                    )
                else:
                    nc.vector.tensor_mul(
                        at[npo : npo + 64, nhp, :], no_sb[0:DH, :], nrb
                    )
                nc.vector.tensor_add(
                    at[npo : npo + 64, nhp, :],
                    at[npo : npo + 64, nhp, :],
                    xt[npo : npo + 64, nhp, :],
                )

            def emit_att(hp, e8):
                va = st[("va", s)]  # created by gen_trans's va section
                if "noatt" in DIAG:
                    for half in range(2):
                        yield
                    return
                for half in range(2):
                    h = 2 * hp + half
                    po = 64 * half
                    po_t = psum.tile([DH + 2, N], f32, name="po_t", tag="pso")
                    for n0, nw in NSPLIT:
                        for i in range(MP):
                            nc.tensor.matmul(
                                po_t[:, n0 : n0 + nw],
                                lhsT=va[:, 2 * i : 2 * i + 2, h, :],
                                rhs=e8[:, half, 2 * i : 2 * i + 2, n0 : n0 + nw],
                                start=(i == 0),
                                stop=(i == MP - 1),
                                perf_mode=DR,
                            )
                    o_sb = rbp.tile([DH + 1, N], bf, name="o_sb", tag="o_sb", bufs=4)
                    nc.vector.tensor_copy(o_sb, po_t[0 : DH + 1, :])
                    if "nonorm" in DIAG:
                        pending_norm.append((hp, po, o_sb, None))
                        if len(pending_norm) > 1:
                            emit_norm()
                        yield
                        continue
                    # InstReciprocal costs ~4.7us on HW; approx_fast (~18
                    # correct bits, one custom-DVE op) is ~5x cheaper and far
                    # exceeds the bf16 precision of the multiply it feeds.
                    # Its fp32 bit-trick seed needs a partition-0 SBUF input:
                    # stage the PSUM denominator row first.
                    den32 = smal.tile([1, N], f32, name="den32", tag="den32", bufs=2)
                    nc.vector.tensor_copy(den32, po_t[DH : DH + 1, :])
                    r_row = smal.tile([1, N], f32, name="r_row", tag="r_row", bufs=3)
                    nc.vector.reciprocal_approx_fast(out=r_row, in_=den32)
                    # broadcast 1/den across 64 partitions on the Pool engine:
                    # no DMA round trip, keeps SP free for bulk loads
                    rb = rbp.tile([64, N], f32, name="rb", tag="rb", bufs=4)
                    nc.gpsimd.partition_broadcast(rb, r_row)
                    # depth-3 norm pipeline: the DVE->Pool->DVE round trip
                    # (recip -> pbcast -> norm-mul) costs ~3.5us in sem-wake
                    # latency on HW; emitting the norm 3 half-heads late keeps
                    # DVE from ever blocking on rb
                    pending_norm.append((hp, po, o_sb, rb))
                    if len(pending_norm) > 3:
                        emit_norm()
                    yield

            # software-pipelined: scores of pair hp, then att of pair hp-1.
            # For the final unit there is no later exp work to hide under, so
            # run att eagerly right after each pair's scores to shorten the
            # drain tail.
            if last:
                for hp in range(HP):
                    e8 = ep.tile([128, 2, MC, N], f8, name="e", tag="e")
                    yield from emit_scores(hp, e8)
                    yield from emit_att(hp, e8)
            else:
                prev_e8 = None
                for hp in range(HP):
                    e8 = ep.tile([128, 2, MC, N], f8, name="e", tag="e")
                    yield from emit_scores(hp, e8)
                    if prev_e8 is not None:
                        yield from emit_att(hp - 1, prev_e8)
                    prev_e8 = e8
                yield from emit_att(HP - 1, prev_e8)
            while pending_norm:
                emit_norm()

        def gen_proj(st, s, b):
            at = st[("at", s)]
            if "noproj" in DIAG or "noatt" in DIAG:
                for oc in range(OC):
                    yield
                return
            for oc in range(OC):
                pu = psum.tile([128, N], f32, name="pu", tag="psx" if oc % 2 else "pso")
                for kt in range(KT):
                    for n0, nw in NSPLIT:
                        nc.tensor.matmul(
                            pu[:, n0 : n0 + nw],
                            lhsT=wo_sb[s][:, kt, oc * 128 : (oc + 1) * 128],
                            rhs=at[:, kt, n0 : n0 + nw],
                            start=(kt == 0),
                            stop=(kt == KT - 1),
                        )
                u_sb = up.tile([128, N], f32, name="u", tag="u")
                if skip_b_o:
                    nc.vector.tensor_copy(u_sb, pu)
                else:
                    nc.vector.tensor_scalar_add(u_sb, pu, bo_sb[s][:, oc : oc + 1])
                nc.sync.dma_start(out=out_d[s, b, oc], in_=u_sb)
                yield

        def drain(g):
            if g is not None:
                for _ in g:
                    pass

        units = [(r, bb, s) for r in range(reps) for bb in range(BPC) for s in range(2)]
        states = {}

        def state_for(r, bb):
            return states.setdefault((r, bb), {})

        from itertools import islice

        st0 = state_for(units[0][0], units[0][1])
        drain(gen_prep(units[0][0], units[0][1], st0))
        # stream unit-0's trans: emit the first two k/qr chunks (pair-0
        # scores inputs), leave the rest as a filler inside its head loop
        tr0 = gen_trans(st0, units[0][2])
        for _ in islice(tr0, 3):
            pass

        pending_proj = None
        pending_heads = {}  # unit index -> (generator, yields already consumed)
        for i, (r, bb, s) in enumerate(units):
            st = state_for(r, bb)
            fillers = []
            if i == 0:
                fillers.append(tr0)
            if pending_proj is not None:
                fillers.append(pending_proj)
            nxt_heads = None
            pre = [0]
            if i + 1 < len(units):
                rn, bn, sn = units[i + 1]
                stn = state_for(rn, bn)
                if (rn, bn) != (r, bb):
                    fillers.append(gen_prep(rn, bn, stn))
                fillers.append(gen_trans(stn, sn))
                # cross-unit head overlap: after the next unit's trans/va
                # fillers drain, let its first score chunks emit inside THIS
                # unit's head loop so ACT's exp stream never drains at the
                # unit boundary

                def counted(g, cnt):
                    for x in g:
                        cnt[0] += 1
                        yield x

                nxt_heads = gen_heads(stn, sn, last=(i + 1 == len(units) - 1))
                fillers.append(islice(counted(nxt_heads, pre), 16))
            heads, done = pending_heads.pop(i, (None, 0))
            if heads is None:
                heads = gen_heads(st, s)
            nyield = HP * MC + H  # score-chunk yields + att-half yields
            for h in range(nyield - done):
                if next(heads, StopIteration) is StopIteration:
                    break
                # drip-feed fillers: 1/yield early so serialized trans/proj
                # drains never dam up the PE stream ahead of score matmuls,
                # 2/yield later to finish the supply before this unit ends
                for _ in range(1 if h < 12 else 2):
                    while fillers:
                        try:
                            next(fillers[0])
                            break
                        except StopIteration:
                            fillers.pop(0)
                    else:
                        break
            drain(heads)
            for g in fillers:
                drain(g)
            if nxt_heads is not None:
                pending_heads[i + 1] = (nxt_heads, pre[0])
            pending_proj = gen_proj(st, s, bb)
        drain(pending_proj)

    nc.finalize()
    return nc


def _prep_inputs(inputs):
    bf16 = ml_dtypes.bfloat16
    fp8 = ml_dtypes.float8_e4m3
    f32 = np.float32

    def arr(name):
        return np.asarray(inputs[name], f32)

    v, q = arr("v"), arr("q")
    v_mask, q_mask = arr("v_mask"), arr("q_mask")

    def prep_x(x, dtype):  # [B, N, D] -> [B, KT, 128, N] (transposed)
        xt = np.ascontiguousarray(x.transpose(0, 2, 1))
        return xt.reshape(B, KT, 128, N).astype(dtype)

    def prep_w(w, dtype, scale=1.0):  # [F, D] -> [KT, 128, F]  (= w.T tiled)
        wt = np.ascontiguousarray(w.T) * f32(scale)
        return wt.reshape(KT, 128, -1).astype(dtype)

    def col128(bias, scale=1.0):  # [F] -> [128, F//128] f32 columns
        return np.ascontiguousarray((bias * f32(scale)).reshape(-1, 128).T).astype(f32)

    w_v, w_q = arr("w_v"), arr("w_q")
    b_v, b_q = arr("b_v"), arr("b_q")
    w_q4v, w_v4q = arr("w_q4v"), arr("w_v4q")
    b_q4v, b_v4q = arr("b_q4v"), arr("b_v4q")
    w_vo, w_qo = arr("w_vo"), arr("w_qo")
    b_vo, b_qo = arr("b_vo"), arr("b_qo")

    xT = np.stack([prep_x(v, bf16), prep_x(q, bf16)])  # [2, B, KT, 128, N]
    x8 = np.stack([prep_x(v, fp8), prep_x(q, fp8)])
    wkq = np.stack(
        [prep_w(w_v[: 2 * OUT], fp8, WSCALE), prep_w(w_q[: 2 * OUT], fp8, WSCALE)]
    )
    wva = np.stack(
        [prep_w(w_v[2 * OUT :], fp8, WSCALE), prep_w(w_q[2 * OUT :], fp8, WSCALE)]
    )
    wg = np.stack([prep_w(w_q4v, bf16), prep_w(w_v4q, bf16)])  # stream 0 gated via q_mean
    wo = np.stack([prep_w(w_vo, bf16), prep_w(w_qo, bf16)])
    bkq = np.stack(
        [col128(b_v[: 2 * OUT], WSCALE), col128(b_q[: 2 * OUT], WSCALE)]
    )
    bva = np.stack(
        [b_v[2 * OUT :][None, :], b_q[2 * OUT :][None, :]]
    ).astype(f32) * f32(WSCALE)
    bgc = np.stack([col128(b_q4v), col128(b_v4q)])
    bgr = np.stack([b_q4v[None, :], b_v4q[None, :]]).astype(f32)
    bo = np.stack([col128(b_vo), col128(b_qo)])

    rms_v = -1.0 / v_mask.sum(1)  # [B]; negative: kernel computes exp(-z)
    rms_q = -1.0 / q_mask.sum(1)
    rms = np.empty((2, B, 128, 1), f32)
    rms[0] = np.broadcast_to(rms_v[:, None, None], (B, 128, 1))
    rms[1] = np.broadcast_to(rms_q[:, None, None], (B, 128, 1))

    skips = (
        bool((b_v[: 2 * OUT] == 0).all() and (b_q[: 2 * OUT] == 0).all()),
        bool((b_v[2 * OUT :] == 0).all() and (b_q[2 * OUT :] == 0).all()),
        bool((b_q4v == 0).all() and (b_v4q == 0).all()),
        bool((b_vo == 0).all() and (b_qo == 0).all()),
    )

    in_maps = []
    for c in range(NCORES):
        sl = slice(c * BPC, (c + 1) * BPC)
        in_maps.append(
            {
                "xT": np.ascontiguousarray(xT[:, sl]),
                "x8": np.ascontiguousarray(x8[:, sl]),
                "wkq": wkq,
                "wva": wva,
                "wg": wg,
                "wo": wo,
                "bkq": bkq,
                "bva": bva,
                "bgc": bgc,
                "bgr": bgr,
                "bo": bo,
                "rms": np.ascontiguousarray(rms[:, sl]),
            }
        )
    return in_maps, skips


def _get_program(skips, reps=1):
    import os

    key = ("prog", skips, reps, os.environ.get("KDIAG", ""))
    if key not in _CACHE:
        _CACHE[key] = _build_program(*skips, reps=reps)
    return _CACHE[key]


def kernel(trace=False, **inputs):
    from concourse.bass_utils import run_bass_kernel_spmd

    in_maps, skips = _prep_inputs(inputs)
    nc = _get_program(skips)
    res = run_bass_kernel_spmd(
        nc, in_maps, core_ids=list(range(NCORES)), trace=trace
    )
    _CACHE["last_result"] = res
    outs = np.stack([r["out"] for r in res.results])  # [8, 2, BPC, OC, 128, N]
    u = outs.reshape(NCORES, 2, BPC, D, N)
    uv = u[:, 0].reshape(B, D, N).transpose(0, 2, 1)
    uq = u[:, 1].reshape(B, D, N).transpose(0, 2, 1)
    return (
        np.ascontiguousarray(uv).astype(np.float32),
        np.ascontiguousarray(uq).astype(np.float32),
    )



# revision 30
# speedup vs baseline: 1.4160x; 1.1671x over previous
"""Trainium2 Bass kernel for DyIntraModalityUpdate (dual gated self-attention).

Strategy
--------
Data-parallel over batch: 16 batches -> 8 NeuronCores x 2 batches, zero
collectives.  Each core processes 4 independent "units" (2 batches x
{v-stream, q-stream}); the only cross-stream coupling is the gates
(v_mean gates q's attention and vice versa), computed per batch before the
per-stream work.

All heavy compute is done in a transposed layout [feature, position]:
  - k/qr projections are computed directly transposed: kqrT[f, r], via
    fp8e4m3 DoubleRow matmuls (2 k-tiles of 128 contracted per instruction,
    ~1.5-2x PE throughput).  Weights are pre-scaled x16 on the host so fp8
    values avoid the subnormal range; the resulting 256x score scale is
    folded into the softmax exp scale.
  - scores: per head pair (2k, 2k+1) the S^T matmuls are emitted
    interleaved; the two heads' lhsT/rhs live at partitions 0-63 / 64-127,
    so the PE row-tiling (tile_position rows 0 and 64) runs both heads'
    matmuls concurrently (~2x).
  - E^T = exp(S^T * 0.125/256) written as fp8e4m3 (attention here is very
    flat - probs ~ 1/768 - so fp8 quantization noise averages out).
  - va is computed in natural layout [position, feature] (fp8 DoubleRow),
    gated, stored as fp8 with a 16.0-column appended so the att-out
    matmul's extra output row yields 16x the softmax denominator
    (compensating the x16 va scale exactly after the reciprocal).
  - att-out O^T = va_ext^T @ E^T via fp8 DoubleRow over position-tile
    pairs.
  - normalization multiplies O^T rows by 1/denominator: the denominator row
    is staged to SBUF f32, inverted with reciprocal_approx_fast (the plain
    InstReciprocal costs ~4.7us on HW), broadcast across 64 partitions with
    gpsimd.partition_broadcast (no DMA round trip), and applied 3 half-heads
    later (depth-3 norm pipeline) so DVE never blocks on the Pool hop.
  - residual add on DVE; the Pool engine runs ONLY partition_broadcast:
    mixing gpsimd op families reloads the Q7 overlay (~8us per switch).
  - final projection stays bf16 (precision: the residual feeds the output
    directly).

HW notes (measured, CoreSim's model differs): exp [128,768] from PSUM is
~0.66us and the ACT stream is NOT the wall-clock pacer; fp8 DoubleRow
matmuls run at ~1 row/cycle (only the halved pass count helps); DVE psum
copies ~0.54us; InstReciprocal ~4.7us regardless of partition count;
cross-engine chains cost ~1us per semaphore wake, so every per-head
dependency is pipelined at least 2-3 heads deep.

Problem constants are hardcoded per the harness contract.
"""

import numpy as np
import ml_dtypes

B, N, D, OUT, H, DH = 16, 768, 512, 512, 8, 64
NCORES, BPC = 8, 2
KT = D // 128          # 4 contraction tiles of 128
KP = KT // 2           # 2 DoubleRow pair-tiles
FC_KQR = (2 * OUT) // 128   # 8 feature chunks for k+qr
OC = OUT // 128        # 4 output chunks
MC = N // 128          # 6 position chunks
MP = MC // 2           # 3 DoubleRow position pairs
HP = H // 2            # 4 head pairs
NSPLIT = ((0, 512), (512, 256))   # psum free-dim splits (bank aligned)
WSCALE = 16.0          # host-side fp8 weight prescale (avoids subnormals)
ESCALE = 0.125 / (WSCALE * WSCALE)  # exp scale absorbing k,qr prescale

_CACHE = {}


def _build_program(skip_b_kq, skip_b_va, skip_b_g, skip_b_o, reps=1):
    import os
    from contextlib import ExitStack

    DIAG = frozenset(
        x for x in os.environ.get("KDIAG", "").split(",") if x
    )  # timing-only ablations; breaks numerics

    import concourse.bass as bass
    import concourse.mybir as mybir
    import concourse.tile as tile
    from concourse import bacc

    dt = mybir.dt
    f32, bf, f8 = dt.float32, dt.bfloat16, dt.float8e4
    AF = mybir.ActivationFunctionType
    OP = mybir.AluOpType
    DR = mybir.MatmulPerfMode.DoubleRow

    nc = bacc.Bacc("TRN2", target_bir_lowering=False, debug=False)

    # ---- DRAM parameters (per-core shard) -------------------------------
    xT_d = nc.declare_dram_parameter("xT", [2, BPC, KT, 128, N], bf, isOutput=False)
    x8_d = nc.declare_dram_parameter("x8", [2, BPC, KT, 128, N], f8, isOutput=False)
    wkq_d = nc.declare_dram_parameter("wkq", [2, KT, 128, 2 * OUT], f8, isOutput=False)
    wva_d = nc.declare_dram_parameter("wva", [2, KT, 128, OUT], f8, isOutput=False)
    wg_d = nc.declare_dram_parameter("wg", [2, KT, 128, OUT], bf, isOutput=False)
    wo_d = nc.declare_dram_parameter("wo", [2, KT, 128, OUT], bf, isOutput=False)
    bkq_d = nc.declare_dram_parameter("bkq", [2, 128, FC_KQR], f32, isOutput=False)
    bva_d = nc.declare_dram_parameter("bva", [2, 1, OUT], f32, isOutput=False)
    bgc_d = nc.declare_dram_parameter("bgc", [2, 128, OC], f32, isOutput=False)
    bgr_d = nc.declare_dram_parameter("bgr", [2, 1, OUT], f32, isOutput=False)
    bo_d = nc.declare_dram_parameter("bo", [2, 128, OC], f32, isOutput=False)
    rms_d = nc.declare_dram_parameter("rms", [2, BPC, 128, 1], f32, isOutput=False)
    out_d = nc.declare_dram_parameter("out", [2, BPC, OC, 128, N], f32, isOutput=True)

    with ExitStack() as ctx:
        tc = ctx.enter_context(tile.TileContext(nc))

        const = ctx.enter_context(tc.tile_pool(name="const", bufs=1))
        xpool = ctx.enter_context(tc.tile_pool(name="xp", bufs=4))
        x8pool = ctx.enter_context(tc.tile_pool(name="x8p", bufs=4))
        kqrp = ctx.enter_context(tc.tile_pool(name="kqrp", bufs=2))
        vap = ctx.enter_context(tc.tile_pool(name="vap", bufs=2))
        ep = ctx.enter_context(tc.tile_pool(name="ep", bufs=3))
        atp = ctx.enter_context(tc.tile_pool(name="atp", bufs=3))
        smal = ctx.enter_context(tc.tile_pool(name="smal", bufs=4))
        up = ctx.enter_context(tc.tile_pool(name="up", bufs=3))
        rbp = ctx.enter_context(tc.tile_pool(name="rbp", bufs=3))
        # PSUM: 8 banks.  "pss" 2x[128,768] (4 banks) rotate the score
        # chunks PE->ACT; "pso" 1x (2 banks) holds the att-out accumulator;
        # "psx" 1x (2 banks) serves trans/va/proj/gate matmuls.
        psum = ctx.enter_context(tc.tile_pool(name="psum", bufs=1, space="PSUM"))

        # ---- batch-0 activations first ----------------------------------
        # stream 1 first: the first gate (s=0) needs stream 1's mean, so its
        # x load and reduces lead the startup critical path.  x8 loads go on
        # the ACT hwdge queue so they don't queue behind the bf16 loads.
        x_first, x8_first = [None, None], [None, None]
        for s in (1, 0):
            xt = xpool.tile([128, KT, N], bf, name="x", tag="x")
            nc.sync.dma_start(out=xt, in_=xT_d[s, 0].rearrange("t p n -> p t n"))
            x_first[s] = xt
        for s in (0, 1):
            x8 = x8pool.tile([128, KT, N], f8, name="x8", tag="x8")
            nc.scalar.dma_start(out=x8, in_=x8_d[s, 0].rearrange("t p n -> p t n"))
            x8_first[s] = x8

        rms_all = {}
        for bb in range(BPC):
            for s in range(2):
                rt = const.tile([128, 1], f32, name=f"rms{s}_{bb}")
                nc.sync.dma_start(out=rt, in_=rms_d[s, bb])
                rms_all[(s, bb)] = rt

        # ---- load weights / biases once ---------------------------------
        wkq_sb, wva_sb, wg_sb, wo_sb = [], [], [], []
        bkq_sb, bgc_sb, bo_sb, bva_sb, bgr_sb = [], [], [], [], []
        bgcn_sb, bgrn_sb = [], []
        for s in range(2):
            t_kq = const.tile([128, KT, 2 * OUT], f8, name=f"wkq{s}")
            t_va = const.tile([128, KT, OUT], f8, name=f"wva{s}")
            t_g = const.tile([128, KT, OUT], bf, name=f"wg{s}")
            t_o = const.tile([128, KT, OUT], bf, name=f"wo{s}")
            wkq_sb.append(t_kq)
            wva_sb.append(t_va)
            wg_sb.append(t_g)
            wo_sb.append(t_o)
        for s in range(2):
            if not skip_b_kq:
                t = const.tile([128, FC_KQR], f32, name=f"bkq{s}")
                nc.sync.dma_start(out=t, in_=bkq_d[s])
                bkq_sb.append(t)
            else:
                bkq_sb.append(None)
            if not skip_b_g:
                t = const.tile([128, OC], f32, name=f"bgc{s}")
                nc.sync.dma_start(out=t, in_=bgc_d[s])
                bgc_sb.append(t)
                tn = const.tile([128, OC], f32, name=f"bgcn{s}")
                nc.vector.tensor_scalar_mul(tn, t, -1.0)
                bgcn_sb.append(tn)
                t = const.tile([1, OUT], f32, name=f"bgr{s}")
                nc.sync.dma_start(out=t, in_=bgr_d[s])
                bgr_sb.append(t)
                tn = const.tile([1, OUT], f32, name=f"bgrn{s}")
                nc.vector.tensor_scalar_mul(tn, t, -1.0)
                bgrn_sb.append(tn)
            else:
                bgc_sb.append(None)
                bgcn_sb.append(None)
                bgr_sb.append(None)
                bgrn_sb.append(None)
            if not skip_b_o:
                t = const.tile([128, OC], f32, name=f"bo{s}")
                nc.sync.dma_start(out=t, in_=bo_d[s])
                bo_sb.append(t)
            else:
                bo_sb.append(None)
            if not skip_b_va:
                t = const.tile([1, OUT], f32, name=f"bva{s}")
                nc.sync.dma_start(out=t, in_=bva_d[s])
                bva_sb.append(t)
            else:
                bva_sb.append(None)
        ident11 = const.tile([1, 1], f32, name="ident11")
        nc.vector.memset(ident11, 1.0)
        if "nogate" in DIAG:
            const_g2 = const.tile([128, OC], f32, name="cg2")
            nc.vector.memset(const_g2, 1.0)
            const_G = const.tile([128, OUT], bf, name="cG")
            nc.vector.memset(const_G, 1.0)
        nc.gpsimd.dma_start(out=wkq_sb[0], in_=wkq_d[0].rearrange("t p f -> p t f"))
        nc.gpsimd.dma_start(out=wg_sb[0], in_=wg_d[0].rearrange("t p f -> p t f"))
        nc.gpsimd.dma_start(out=wg_sb[1], in_=wg_d[1].rearrange("t p f -> p t f"))
        nc.gpsimd.dma_start(out=wva_sb[0], in_=wva_d[0].rearrange("t p f -> p t f"))
        nc.gpsimd.dma_start(out=wkq_sb[1], in_=wkq_d[1].rearrange("t p f -> p t f"))
        nc.sync.dma_start(out=wva_sb[1], in_=wva_d[1].rearrange("t p f -> p t f"))
        nc.sync.dma_start(out=wo_sb[0], in_=wo_d[0].rearrange("t p f -> p t f"))
        nc.sync.dma_start(out=wo_sb[1], in_=wo_d[1].rearrange("t p f -> p t f"))

        # ---- interleaved per-unit emission ------------------------------

        def gen_prep(rep_i, b, st):
            if rep_i == 0 and b == 0:
                st["x"], st["x8"] = x_first, x8_first
            else:
                st["x"], st["x8"] = [], []
                # steady-state x loads go on the ACT hwdge queue: the SP
                # queue carries the out stores, which would delay the means
                # -> gates chain that the next unit's k-gating waits on
                for s in range(2):
                    xt = xpool.tile([128, KT, N], bf, name="x", tag="x")
                    nc.scalar.dma_start(
                        out=xt, in_=xT_d[s, b].rearrange("t p n -> p t n")
                    )
                    st["x"].append(xt)
                for s in range(2):
                    x8 = x8pool.tile([128, KT, N], f8, name="x8", tag="x8")
                    nc.scalar.dma_start(
                        out=x8, in_=x8_d[s, b].rearrange("t p n -> p t n")
                    )
                    st["x8"].append(x8)
            yield
            if "nogate" in DIAG:
                yield
                st["gcol"] = [const_g2, const_g2]
                st["G"] = [const_G, const_G]
                return
            x_sb = st["x"]
            mean_sb, rms_sb = {}, {}
            for s in (1, 0):  # stream 1 first: gate s=0 needs mean of 1
                rms_sb[s] = rms_all[(s, b)]
                sums = smal.tile([128, KT], f32, name="sums", tag="sums")
                for kt in range(KT):
                    nc.vector.reduce_sum(
                        out=sums[:, kt : kt + 1],
                        in_=x_sb[s][:, kt, :],
                        axis=mybir.AxisListType.X,
                    )
                mean = smal.tile([128, KT], bf, name="mean", tag="mean")
                nc.vector.tensor_copy(mean, sums)
                mean_sb[s] = mean
            yield
            # row-only gate computation: the per-oc column path (16 tiny PE
            # matmuls + 4 ACT exps + 4 DVE ops per stream) cost ~1us per
            # cross-engine hop on HW.  Compute g and g^2 as rows, then turn
            # g^2 into per-partition columns with one DRAM round trip per
            # batch (latency hidden: prep runs a unit ahead).
            gcol_sb, G_sb = [], []
            for s in range(2):
                o = 1 - s
                # sigmoid via exp (stay in ACT's exp table set): rms_d
                # carries -1/mask_sum, so e = exp(-z) and g = 1 + 1/(1+e)
                pr = psum.tile([1, OUT], f32, name="pr", tag="psx")
                for kt in range(KT):
                    nc.tensor.matmul(
                        pr,
                        lhsT=mean_sb[o][:, kt : kt + 1],
                        rhs=wg_sb[s][:, kt, :],
                        start=(kt == 0),
                        stop=(kt == KT - 1),
                    )
                sig_r = smal.tile([1, OUT], f32, name="sig_r", tag="sig_r", bufs=2)
                if skip_b_g:
                    nc.scalar.activation(
                        out=sig_r, in_=pr, func=AF.Exp, scale=rms_sb[o][0:1, :]
                    )
                else:
                    tmp_r = smal.tile([1, OUT], f32, name="tmp_r", tag="tmp_r", bufs=2)
                    nc.vector.scalar_tensor_tensor(
                        out=tmp_r,
                        in0=pr,
                        scalar=rms_sb[o][0:1, :],
                        in1=bgrn_sb[s],
                        op0=OP.mult,
                        op1=OP.add,
                    )
                    nc.scalar.activation(out=sig_r, in_=tmp_r, func=AF.Exp)
                t1r = smal.tile([1, OUT], f32, name="t1r", tag="t1r", bufs=2)
                nc.vector.tensor_scalar_add(t1r, sig_r, 1.0)
                rr = smal.tile([1, OUT], f32, name="rr", tag="rr", bufs=2)
                nc.vector.reciprocal_approx_fast(out=rr, in_=t1r)
                grow = smal.tile([1, OUT], bf, name="grow", tag="grow", bufs=2)
                nc.vector.tensor_scalar_add(grow, rr, 1.0)
                G = rbp.tile([128, OUT], bf, name="G", tag="G", bufs=2)
                nc.gpsimd.partition_broadcast(G, grow)
                G_sb.append(G)
                g2row = smal.tile([1, OUT], f32, name="g2row", tag="g2row", bufs=2)
                nc.vector.tensor_mul(g2row, grow, grow)
                # turn the g^2 row into per-partition columns with 4 tiny PE
                # transposes (no DMA, no extra engine hops)
                pgt = psum.tile([128, OC], f32, name="pgt", tag="psx")
                for oc in range(OC):
                    nc.tensor.transpose(
                        pgt[:, oc : oc + 1],
                        g2row[0:1, oc * 128 : (oc + 1) * 128],
                        ident11,
                    )
                g2c = smal.tile([128, OC], f32, name="g2c", tag="g2c", bufs=2)
                nc.vector.tensor_copy(g2c, pgt)
                gcol_sb.append(g2c)
                yield
            st["gcol"], st["G"] = gcol_sb, G_sb

        def gen_trans(st, s):
            x8 = st["x8"][s]
            gcol_sb = st["gcol"]
            kqr = kqrp.tile([128, FC_KQR, N], bf, name="kqr", tag="kqr")
            st[("kqr", s)] = kqr
            # ungated qr chunks (fc >= OC) first: their drains don't wait on
            # the gate chain, giving it ~4 more yield-slots to resolve before
            # a gated k-chunk drain can head-of-line-block the DVE queue.
            for fi, fc in enumerate((OC, OC + 1, 0, 1, OC + 2, 2, OC + 3, 3)):
                pt = psum.tile([128, N], f32, name="pt", tag="psx" if fi % 2 else "pso")
                for i in range(KP):
                    for n0, nw in NSPLIT:
                        nc.tensor.matmul(
                            pt[:, n0 : n0 + nw],
                            lhsT=wkq_sb[s][:, 2 * i : 2 * i + 2, fc * 128 : (fc + 1) * 128],
                            rhs=x8[:, 2 * i : 2 * i + 2, n0 : n0 + nw],
                            start=(i == 0),
                            stop=(i == KP - 1),
                            perf_mode=DR,
                        )
                if fc < OC:
                    gsl = gcol_sb[s][:, fc : fc + 1]
                    if skip_b_kq:
                        nc.vector.tensor_scalar_mul(kqr[:, fc, :], pt, gsl)
                    else:
                        bg2 = smal.tile([128, 1], f32, name="bg2", tag="bg2")
                        nc.vector.tensor_mul(bg2, bkq_sb[s][:, fc : fc + 1], gsl)
                        nc.scalar.activation(
                            out=kqr[:, fc, :],
                            in_=pt,
                            func=AF.Identity,
                            bias=bg2,
                            scale=gsl,
                        )
                else:
                    if skip_b_kq:
                        nc.vector.tensor_copy(kqr[:, fc, :], pt)
                    else:
                        nc.scalar.activation(
                            out=kqr[:, fc, :],
                            in_=pt,
                            func=AF.Identity,
                            bias=bkq_sb[s][:, fc : fc + 1],
                        )
                yield

            va = vap.tile([128, MC, H, DH + 2], f8, name="va", tag="va")
            st[("va", s)] = va
            with nc.allow_low_precision("fp8 attention values"):
                nc.vector.memset(va[:, :, :, DH : DH + 1], WSCALE)
                nc.vector.memset(va[:, :, :, DH + 1 : DH + 2], 0.0)
            G_h = st["G"][s].rearrange("p (h d) -> p h d", h=H)
            for mc in range(MC):
                pv = psum.tile([128, OUT], f32, name="pv", tag="psx" if mc % 2 else "pso")
                for i in range(KP):
                    nc.tensor.matmul(
                        pv,
                        lhsT=x8[:, 2 * i : 2 * i + 2, mc * 128 : (mc + 1) * 128],
                        rhs=wva_sb[s][:, 2 * i : 2 * i + 2, :],
                        start=(i == 0),
                        stop=(i == KP - 1),
                        perf_mode=DR,
                    )
                pv_h = pv.rearrange("p (h d) -> p h d", h=H)
                with nc.allow_low_precision("fp8 attention values"):
                    nc.vector.tensor_mul(va[:, mc, :, 0:DH], pv_h, G_h)
                    if not skip_b_va:
                        bgr_row = smal.tile([1, OUT], f32, name="bgr_row", tag="bgrr")
                        nc.vector.tensor_mul(bgr_row, bva_sb[s], st["G"][s][0:1, :])
                        bg = rbp.tile([128, OUT], f32, name="bg", tag="bg")
                        nc.gpsimd.partition_broadcast(bg, bgr_row)
                        nc.vector.tensor_add(
                            va[:, mc, :, 0:DH],
                            va[:, mc, :, 0:DH],
                            bg.rearrange("p (h d) -> p h d", h=H),
                        )
                yield

        def gen_heads(st, s, last=False):
            xt = st["x"][s]
            kqr = st[("kqr", s)]
            at = atp.tile([128, OC, N], bf, name="at", tag="at")
            st[("at", s)] = at

            def emit_scores(hp, e8):
                # both heads of the pair interleaved at mc granularity:
                # head 2hp on PE rows 0-63, head 2hp+1 on rows 64-127 run
                # concurrently (row tiling).
                for mc in range(MC):
                    for half in range(2):
                        po = 64 * half
                        ps_s = psum.tile([128, N], f32, name="ps_s", tag="pss", bufs=2)
                        lhsT = kqr[po : po + 64, hp, mc * 128 : (mc + 1) * 128]
                        for n0, nw in NSPLIT:
                            nc.tensor.matmul(
                                ps_s[:, n0 : n0 + nw],
                                lhsT=lhsT,
                                rhs=kqr[po : po + 64, OC + hp, n0 : n0 + nw],
                                start=True,
                                stop=True,
                            )
                        with nc.allow_low_precision("fp8 attention probs"):
                            if "tinyexp" in DIAG:
                                nc.scalar.activation(
                                    out=e8[:, half, mc, 0:32],
                                    in_=ps_s[:, 0:32],
                                    func=AF.Exp,
                                    scale=ESCALE,
                                )
                            else:
                                nc.scalar.activation(
                                    out=e8[:, half, mc, :],
                                    in_=ps_s,
                                    func=AF.Exp,
                                    scale=ESCALE,
                                )
                    yield

            pending_norm = []  # (hp, po, o_sb, rb) with broadcast in flight

            def emit_norm():
                # at-mul + residual for the oldest pending head; its rb
                # broadcast has been in flight while the next head's att
                # matmuls ran, so DVE never waits on the DMA round trip.
                # residual add on DVE (all-SBUF bf16): Pool must stay
                # pbcast-only — mixing gpsimd op families reloads the Q7
                # overlay per switch, serializing the whole norm chain
                nhp, npo, no_sb, nrb = pending_norm.pop(0)
                if nrb is None:  # KDIAG=nonorm: same DVE volume, no rb dep
                    nc.vector.tensor_copy(
                        at[npo : npo + 64, nhp, :], no_sb[0:DH, :]
                    )
                else:
                    nc.vector.tensor_mul(
                        at[npo : npo + 64, nhp, :], no_sb[0:DH, :], nrb
                    )
                nc.vector.tensor_add(
                    at[npo : npo + 64, nhp, :],
                    at[npo : npo + 64, nhp, :],
                    xt[npo : npo + 64, nhp, :],
                )

            def emit_att(hp, e8):
                va = st[("va", s)]  # created by gen_trans's va section
                if "noatt" in DIAG:
                    for half in range(2):
                        yield
                    return
                for half in range(2):
                    h = 2 * hp + half
                    po = 64 * half
                    po_t = psum.tile([DH + 2, N], f32, name="po_t", tag="pso")
                    for n0, nw in NSPLIT:
                        for i in range(MP):
                            nc.tensor.matmul(
                                po_t[:, n0 : n0 + nw],
                                lhsT=va[:, 2 * i : 2 * i + 2, h, :],
                                rhs=e8[:, half, 2 * i : 2 * i + 2, n0 : n0 + nw],
                                start=(i == 0),
                                stop=(i == MP - 1),
                                perf_mode=DR,
                            )
                    o_sb = rbp.tile([DH + 1, N], bf, name="o_sb", tag="o_sb", bufs=4)
                    nc.vector.tensor_copy(o_sb, po_t[0 : DH + 1, :])
                    if "nonorm" in DIAG:
                        pending_norm.append((hp, po, o_sb, None))
                        if len(pending_norm) > 1:
                            emit_norm()
                        yield
                        continue
                    # InstReciprocal costs ~4.7us on HW; approx_fast (~18
                    # correct bits, one custom-DVE op) is ~5x cheaper and far
                    # exceeds the bf16 precision of the multiply it feeds.
                    # Its fp32 bit-trick seed needs a partition-0 SBUF input:
                    # stage the PSUM denominator row first.
                    den32 = smal.tile([1, N], f32, name="den32", tag="den32", bufs=2)
                    nc.vector.tensor_copy(den32, po_t[DH : DH + 1, :])
                    r_row = smal.tile([1, N], f32, name="r_row", tag="r_row", bufs=3)
                    nc.vector.reciprocal_approx_fast(out=r_row, in_=den32)
                    # broadcast 1/den across 64 partitions on the Pool engine:
                    # no DMA round trip, keeps SP free for bulk loads
                    rb = rbp.tile([64, N], f32, name="rb", tag="rb", bufs=4)
                    nc.gpsimd.partition_broadcast(rb, r_row)
                    # depth-3 norm pipeline: the DVE->Pool->DVE round trip
                    # (recip -> pbcast -> norm-mul) costs ~3.5us in sem-wake
                    # latency on HW; emitting the norm 3 half-heads late keeps
                    # DVE from ever blocking on rb
                    pending_norm.append((hp, po, o_sb, rb))
                    if len(pending_norm) > 2:
                        emit_norm()
                    yield

            # software-pipelined: scores of pair hp, then att of pair hp-1.
            # For the final unit there is no later exp work to hide under, so
            # run att eagerly right after each pair's scores to shorten the
            # drain tail.
            if last:
                for hp in range(HP):
                    e8 = ep.tile([128, 2, MC, N], f8, name="e", tag="e")
                    yield from emit_scores(hp, e8)
                    yield from emit_att(hp, e8)
            else:
                prev_e8 = None
                for hp in range(HP):
                    e8 = ep.tile([128, 2, MC, N], f8, name="e", tag="e")
                    yield from emit_scores(hp, e8)
                    if prev_e8 is not None:
                        yield from emit_att(hp - 1, prev_e8)
                    prev_e8 = e8
                yield from emit_att(HP - 1, prev_e8)
            while pending_norm:
                emit_norm()

        def gen_proj(st, s, b):
            at = st[("at", s)]
            if "noproj" in DIAG or "noatt" in DIAG:
                for oc in range(OC):
                    yield
                return
            for oc in range(OC):
                pu = psum.tile([128, N], f32, name="pu", tag="psx" if oc % 2 else "pso")
                for kt in range(KT):
                    for n0, nw in NSPLIT:
                        nc.tensor.matmul(
                            pu[:, n0 : n0 + nw],
                            lhsT=wo_sb[s][:, kt, oc * 128 : (oc + 1) * 128],
                            rhs=at[:, kt, n0 : n0 + nw],
                            start=(kt == 0),
                            stop=(kt == KT - 1),
                        )
                u_sb = up.tile([128, N], f32, name="u", tag="u")
                if skip_b_o:
                    nc.vector.tensor_copy(u_sb, pu)
                else:
                    nc.vector.tensor_scalar_add(u_sb, pu, bo_sb[s][:, oc : oc + 1])
                nc.sync.dma_start(out=out_d[s, b, oc], in_=u_sb)
                yield

        def drain(g):
            if g is not None:
                for _ in g:
                    pass

        units = [(r, bb, s) for r in range(reps) for bb in range(BPC) for s in range(2)]
        states = {}

        def state_for(r, bb):
            return states.setdefault((r, bb), {})

        from itertools import islice

        st0 = state_for(units[0][0], units[0][1])
        drain(gen_prep(units[0][0], units[0][1], st0))
        # stream unit-0's trans: emit the first two k/qr chunks (pair-0
        # scores inputs), leave the rest as a filler inside its head loop
        tr0 = gen_trans(st0, units[0][2])
        for _ in islice(tr0, 3):
            pass

        pending_proj = None
        pending_heads = {}  # unit index -> (generator, yields already consumed)
        for i, (r, bb, s) in enumerate(units):
            st = state_for(r, bb)
            fillers = []
            if i == 0:
                fillers.append(tr0)
            if pending_proj is not None:
                fillers.append(pending_proj)
            nxt_heads = None
            pre = [0]
            if i + 1 < len(units):
                rn, bn, sn = units[i + 1]
                stn = state_for(rn, bn)
                if (rn, bn) != (r, bb):
                    fillers.append(gen_prep(rn, bn, stn))
                fillers.append(gen_trans(stn, sn))
                # cross-unit head overlap: after the next unit's trans/va
                # fillers drain, let its first score chunks emit inside THIS
                # unit's head loop so ACT's exp stream never drains at the
                # unit boundary

                def counted(g, cnt):
                    for x in g:
                        cnt[0] += 1
                        yield x

                nxt_heads = gen_heads(stn, sn, last=(i + 1 == len(units) - 1))
                fillers.append(islice(counted(nxt_heads, pre), 16))
            heads, done = pending_heads.pop(i, (None, 0))
            if heads is None:
                heads = gen_heads(st, s)
            nyield = HP * MC + H  # score-chunk yields + att-half yields
            for h in range(nyield - done):
                if next(heads, StopIteration) is StopIteration:
                    break
                # drip-feed fillers: 1/yield early so serialized trans/proj
                # drains never dam up the PE stream ahead of score matmuls,
                # 2/yield later to finish the supply before this unit ends
                for _ in range(1 if h < 12 else 2):
                    while fillers:
                        try:
                            next(fillers[0])
                            break
                        except StopIteration:
                            fillers.pop(0)
                    else:
                        break
            drain(heads)
            for g in fillers:
                drain(g)
            if nxt_heads is not None:
                pending_heads[i + 1] = (nxt_heads, pre[0])
            pending_proj = gen_proj(st, s, bb)
        drain(pending_proj)

    nc.finalize()
    return nc


def _prep_inputs(inputs):
    bf16 = ml_dtypes.bfloat16
    fp8 = ml_dtypes.float8_e4m3
    f32 = np.float32

    def arr(name):
        return np.asarray(inputs[name], f32)

    v, q = arr("v"), arr("q")
    v_mask, q_mask = arr("v_mask"), arr("q_mask")

    def prep_x(x, dtype):  # [B, N, D] -> [B, KT, 128, N] (transposed)
        xt = np.ascontiguousarray(x.transpose(0, 2, 1))
        return xt.reshape(B, KT, 128, N).astype(dtype)

    def prep_w(w, dtype, scale=1.0):  # [F, D] -> [KT, 128, F]  (= w.T tiled)
        wt = np.ascontiguousarray(w.T) * f32(scale)
        return wt.reshape(KT, 128, -1).astype(dtype)

    def col128(bias, scale=1.0):  # [F] -> [128, F//128] f32 columns
        return np.ascontiguousarray((bias * f32(scale)).reshape(-1, 128).T).astype(f32)

    w_v, w_q = arr("w_v"), arr("w_q")
    b_v, b_q = arr("b_v"), arr("b_q")
    w_q4v, w_v4q = arr("w_q4v"), arr("w_v4q")
    b_q4v, b_v4q = arr("b_q4v"), arr("b_v4q")
    w_vo, w_qo = arr("w_vo"), arr("w_qo")
    b_vo, b_qo = arr("b_vo"), arr("b_qo")

    xT = np.stack([prep_x(v, bf16), prep_x(q, bf16)])  # [2, B, KT, 128, N]
    x8 = np.stack([prep_x(v, fp8), prep_x(q, fp8)])
    wkq = np.stack(
        [prep_w(w_v[: 2 * OUT], fp8, WSCALE), prep_w(w_q[: 2 * OUT], fp8, WSCALE)]
    )
    wva = np.stack(
        [prep_w(w_v[2 * OUT :], fp8, WSCALE), prep_w(w_q[2 * OUT :], fp8, WSCALE)]
    )
    wg = np.stack([prep_w(w_q4v, bf16), prep_w(w_v4q, bf16)])  # stream 0 gated via q_mean
    wo = np.stack([prep_w(w_vo, bf16), prep_w(w_qo, bf16)])
    bkq = np.stack(
        [col128(b_v[: 2 * OUT], WSCALE), col128(b_q[: 2 * OUT], WSCALE)]
    )
    bva = np.stack(
        [b_v[2 * OUT :][None, :], b_q[2 * OUT :][None, :]]
    ).astype(f32) * f32(WSCALE)
    bgc = np.stack([col128(b_q4v), col128(b_v4q)])
    bgr = np.stack([b_q4v[None, :], b_v4q[None, :]]).astype(f32)
    bo = np.stack([col128(b_vo), col128(b_qo)])

    rms_v = -1.0 / v_mask.sum(1)  # [B]; negative: kernel computes exp(-z)
    rms_q = -1.0 / q_mask.sum(1)
    rms = np.empty((2, B, 128, 1), f32)
    rms[0] = np.broadcast_to(rms_v[:, None, None], (B, 128, 1))
    rms[1] = np.broadcast_to(rms_q[:, None, None], (B, 128, 1))

    skips = (
        bool((b_v[: 2 * OUT] == 0).all() and (b_q[: 2 * OUT] == 0).all()),
        bool((b_v[2 * OUT :] == 0).all() and (b_q[2 * OUT :] == 0).all()),
        bool((b_q4v == 0).all() and (b_v4q == 0).all()),
        bool((b_vo == 0).all() and (b_qo == 0).all()),
    )

    in_maps = []
    for c in range(NCORES):
        sl = slice(c * BPC, (c + 1) * BPC)
        in_maps.append(
            {
                "xT": np.ascontiguousarray(xT[:, sl]),
                "x8": np.ascontiguousarray(x8[:, sl]),
                "wkq": wkq,
                "wva": wva,
                "wg": wg,
                "wo": wo,
                "bkq": bkq,
                "bva": bva,
                "bgc": bgc,
                "bgr": bgr,
                "bo": bo,
                "rms": np.ascontiguousarray(rms[:, sl]),
            }
        )
    return in_maps, skips


def _get_program(skips, reps=1):
    import os

    key = ("prog", skips, reps, os.environ.get("KDIAG", ""))
    if key not in _CACHE:
        _CACHE[key] = _build_program(*skips, reps=reps)
    return _CACHE[key]


def kernel(trace=False, **inputs):
    from concourse.bass_utils import run_bass_kernel_spmd

    in_maps, skips = _prep_inputs(inputs)
    nc = _get_program(skips)
    res = run_bass_kernel_spmd(
        nc, in_maps, core_ids=list(range(NCORES)), trace=trace
    )
    _CACHE["last_result"] = res
    outs = np.stack([r["out"] for r in res.results])  # [8, 2, BPC, OC, 128, N]
    u = outs.reshape(NCORES, 2, BPC, D, N)
    uv = u[:, 0].reshape(B, D, N).transpose(0, 2, 1)
    uq = u[:, 1].reshape(B, D, N).transpose(0, 2, 1)
    return (
        np.ascontiguousarray(uv).astype(np.float32),
        np.ascontiguousarray(uq).astype(np.float32),
    )

